# revision 11
# baseline (speedup 1.0000x reference)
"""Single-head causal attention on 8 trn2 NeuronCores.

Problem: x[16, 2048, 1024] fp32, Wq/Wk/Wv[1024, 64] fp32 ->
         out[16, 2048, 64] = softmax(causal(q k^T / sqrt(64))) v

Sharding: data-parallel over batch B=16 -> 2 batches per core, no
collectives. SPMD program; the two per-core batches are processed in
lockstep so small-dim matmuls can be packed across them.

Key structure (vs the fp32/f32r baseline this replaces):
  * x is loaded with gpsimd casting DMAs (fp32 HBM -> bf16 SBUF,
    round-to-nearest-even, verified on HW). Everything downstream of
    the load runs in bf16 except PSUM accumulation (always fp32) and
    the final normalize, so PE transposes run at 1 cyc/row and DVE
    copies at the 2-byte 2x rate. Max rel err vs fp64 reference is
    ~4e-3 (gate 2e-2).
  * x^T tiles via PE transpose (8 chunks share one bf16 PSUM bank,
    one wide DVE copy out).
  * Projections: per-batch packed [Wq|Wk] (b0) / [Wk|Wq] (b1) so that
    q0 lands at partitions 0:64 and q1 at 64:128 with plain copies;
    k halves are partition-shifted into a shared k tile by SBUF->SBUF
    DMA. v for both batches in one pass via column tiling
    (tile_position (0,0)/(0,64)), then PE-transposed to natural [T,64]
    with a ones column appended (PV then emits the softmax denominator
    for free).
  * Scores S^T: the two batches' K=64 matmuls run concurrently in the
    PE array via row tiling (tile_position (0,0)/(64,0)) into the two
    halves of one [128,2,512] PSUM pair; exp and causal mask are one
    instruction per j for both batches.
  * Schedule: AB(blk) then C(blk) per T-block, so block k+1's loads/
    transposes overlap block k's attention. The attention j-loop is
    software-pipelined (QK for j+1 is emitted before PV(j)) so the PE
    queue never parks a PV (waiting on exp) ahead of ready score work.
    PSUM is exactly allocated: 4 banks attention ring, 2 banks output
    accumulators, 2 banks load/transpose ring - the finalize transposes
    deliberately use the attention ring, NOT the "ab" ring, so block
    k+1's transposes never wait on block k's finalize.

Cost-model timeline (sim.py): 104.6 us. HW per-rep slope: ~110-118 us
(environment-dependent). Causal mask applies only to the first 128
columns of diagonal tiles (columns >= 128 pass vacuously).
Remaining gap over engine-busy: ~8 us DMA-bound startup, ~5 us serial
finalize tail, ~17 us of per-dependency semaphore latency, and the
exp-volume floor on ACT (~29 us) pacing the causally-last row block.
"""

import sys

sys.path.insert(0, "/opt/trn_rl_repo")

import numpy as np

import concourse.bass as bass  # noqa: F401
import concourse.bacc as bacc
import concourse.mybir as mybir
import concourse.tile as tile
from concourse.masks import make_identity
from concourse.bass_utils import run_bass_kernel_spmd

B, T, C, H = 16, 2048, 1024, 64
NCORES = 8
BPC = B // NCORES  # batches per core
CB = C // 128      # 8 contraction chunks
TT = T // 128      # 16 T tiles of 128
NB = T // 512      # 4 T blocks of 512
F32 = mybir.dt.float32
BF16 = mybir.dt.bfloat16
SCALE = float(H) ** -0.5


def build_program(reps=1, att_bufs=2, ab_bufs=2, pt_bufs=3, x_bufs=3,
                  xt_bufs=2, oex_bufs=2):
    from contextlib import ExitStack

    nc = bacc.Bacc("TRN2", target_bir_lowering=False, debug=False,
                   num_devices=NCORES)
    x_d = nc.dram_tensor("x", [BPC, T, C], F32, kind="ExternalInput").ap()
    wq_d = nc.dram_tensor("Wq", [C, H], F32, kind="ExternalInput").ap()
    wk_d = nc.dram_tensor("Wk", [C, H], F32, kind="ExternalInput").ap()
    wv_d = nc.dram_tensor("Wv", [C, H], F32, kind="ExternalInput").ap()
    y_d = nc.dram_tensor("y", [BPC, T, H], F32, kind="ExternalOutput").ap()

    with tile.TileContext(nc) as tc, ExitStack() as ctx:
        singles = ctx.enter_context(tc.tile_pool(name="singles", bufs=1))
        xp = ctx.enter_context(tc.tile_pool(name="xp", bufs=x_bufs))
        xTp = ctx.enter_context(tc.tile_pool(name="xTp", bufs=xt_bufs))
        qkp = ctx.enter_context(tc.tile_pool(name="qkp", bufs=1))
        vnp = ctx.enter_context(tc.tile_pool(name="vnp", bufs=1))
        vTp = ctx.enter_context(tc.tile_pool(name="vTp", bufs=2))
        ptp = ctx.enter_context(tc.tile_pool(name="ptp", bufs=pt_bufs))
        oxp = ctx.enter_context(tc.tile_pool(name="oxp", bufs=oex_bufs))
        fin = ctx.enter_context(tc.tile_pool(name="fin", bufs=2))
        ps_att = ctx.enter_context(tc.tile_pool(name="psatt", bufs=att_bufs,
                                                space="PSUM"))
        ps_oa = ctx.enter_context(tc.tile_pool(name="psoa", bufs=1,
                                               space="PSUM"))
        ps_ab = ctx.enter_context(tc.tile_pool(name="psab", bufs=ab_bufs,
                                               space="PSUM"))

        # identity build is dependency-free Pool work; run it before the x
        # descriptor generation so the first transposes aren't blocked on it
        with tc.high_priority():
            ident = singles.tile([128, 128], BF16)
            make_identity(nc, ident[:, :])
            identf = singles.tile([128, 128], F32)
            make_identity(nc, identf[:, :])
            # warm the PE array (HAM clock ramp) with dummy matmuls while
            # the first x transfer is in flight; they borrow an attention
            # ring slot that isn't needed until ~13us in
            warm = ps_att.tile([128, BPC, 512], F32, tag="att")
            for i in range(24):
                nc.tensor.matmul(warm[:, i % 2, 0:128], ident[:, :],
                                 ident[:, :], start=True, stop=True)

        # Weights: fp32 staging via HWDGE (keeps Pool free for the x casting
        # DMAs at startup), then convert/pack on gpsimd. b0 packs [Wq|Wk],
        # b1 [Wk|Wq] so q lands on the batch's own row half with a plain
        # copy.
        wq_s = singles.tile([128, CB, 64], F32)
        wk_s = singles.tile([128, CB, 64], F32)
        wv_s = singles.tile([128, CB, 64], F32)
        nc.sync.dma_start(out=wq_s[:, :, :],
                          in_=wq_d.rearrange("(c p) h -> p c h", p=128))
        nc.sync.dma_start(out=wk_s[:, :, :],
                          in_=wk_d.rearrange("(c p) h -> p c h", p=128))
        nc.sync.dma_start(out=wv_s[:, :, :],
                          in_=wv_d.rearrange("(c p) h -> p c h", p=128))
        # packing on Pool: these wait on the weight transfers, and on DVE
        # they would head-of-line block the first transpose copies
        wqk0 = singles.tile([128, CB, 128], BF16)
        nc.gpsimd.tensor_copy(wqk0[:, :, 0:64], wq_s[:, :, :])
        nc.gpsimd.tensor_copy(wqk0[:, :, 64:128], wk_s[:, :, :])
        wqk1 = singles.tile([128, CB, 128], BF16)
        nc.gpsimd.tensor_copy(wqk1[:, :, 0:64], wk_s[:, :, :])
        nc.gpsimd.tensor_copy(wqk1[:, :, 64:128], wq_s[:, :, :])
        wv2 = singles.tile([128, CB, 128], BF16)
        nc.gpsimd.tensor_copy(wv2[:, :, 0:64], wv_s[:, :, :])
        nc.gpsimd.tensor_copy(wv2[:, :, 64:128], wv_s[:, :, :])
        wqk = (wqk0, wqk1)

        def body():
            # q^T for both batches: qk0 rows 0:64 = q0, qk1 rows 64:128 = q1
            qk0 = qkp.tile([128, T], BF16, tag="qk0")
            qk1 = qkp.tile([128, T], BF16, tag="qk1")
            # k^T for both batches: rows 0:64 = k0, rows 64:128 = k1
            skk = qkp.tile([128, T], BF16, tag="skk")
            vn0 = vnp.tile([128, TT, 65], BF16, tag="vn0")
            vn1 = vnp.tile([128, TT, 65], BF16, tag="vn1")
            nc.vector.memset(vn0[:, :, 64], 1.0)
            nc.vector.memset(vn1[:, :, 64], 1.0)
            qkt = (qk0, qk1)
            vnt = (vn0, vn1)

            def phase_ab(blk):
                """Load + transpose x, projections for T block blk."""
                xTs = []
                for b in range(BPC):
                    xT = xTp.tile([128, CB, 512], BF16, tag=f"xT{b}")
                    xTs.append(xT)
                    # one casting DMA per (batch, block): 1 MB fp32 -> bf16.
                    # The very first block is split in two so the transpose
                    # pipeline fills sooner.
                    xt = xp.tile([128, 4, C], BF16, tag="x")
                    base = blk * 512
                    if blk == 0 and b == 0:
                        # first load split in two and boosted ahead of
                        # everything (even the identity build: it hides
                        # under the transfer) so the pipeline fills asap
                        with tc.high_priority(offset=1 << 20):
                            nc.gpsimd.dma_start(
                                out=xt[:, 0:2, :],
                                in_=x_d[b, base:base + 256, :].rearrange(
                                    "(f p) c -> p f c", p=128))
                        with tc.high_priority():
                            nc.gpsimd.dma_start(
                                out=xt[:, 2:4, :],
                                in_=x_d[b, base + 256:base + 512, :].rearrange(
                                    "(f p) c -> p f c", p=128))
                    else:
                        nc.gpsimd.dma_start(
                            out=xt[:, :, :],
                            in_=x_d[b, base:base + 512, :].rearrange(
                                "(f p) c -> p f c", p=128))
                    for t4 in range(4):
                        ptr = ps_ab.tile([128, CB, 128], BF16, tag="ab")
                        for ci in range(CB):
                            nc.tensor.matmul(ptr[:, ci, :],
                                             xt[:, t4, ci * 128:(ci + 1) * 128],
                                             ident[:, :], is_transpose=True)
                        nc.vector.tensor_copy(
                            xT[:, :, t4 * 128:(t4 + 1) * 128], ptr[:, :, :])
                sl = slice(blk * 512, (blk + 1) * 512)
                for b in range(BPC):
                    pq = ps_ab.tile([128, 512], F32, tag="ab")
                    for ci in range(CB):
                        nc.tensor.matmul(pq[:, :], wqk[b][:, ci, :],
                                         xTs[b][:, ci, :],
                                         start=(ci == 0), stop=(ci == CB - 1))
                    nc.vector.tensor_copy(qkt[b][:, sl], pq[:, :])
                # k partition shifts into the shared k tile
                nc.sync.dma_start(out=skk[0:64, sl], in_=qk0[64:128, sl])
                nc.sync.dma_start(out=skk[64:128, sl], in_=qk1[0:64, sl])
                # v for both batches, column-tiled into one PSUM bank
                pv = ps_ab.tile([128, 512], F32, tag="ab")
                for ci in range(CB):
                    nc.tensor.matmul(pv[0:64, :], wv2[:, ci, 0:64],
                                     xTs[0][:, ci, :], start=(ci == 0),
                                     stop=(ci == CB - 1), tile_position=(0, 0))
                    nc.tensor.matmul(pv[64:128, :], wv2[:, ci, 64:128],
                                     xTs[1][:, ci, :], start=(ci == 0),
                                     stop=(ci == CB - 1), tile_position=(0, 64))
                vT = vTp.tile([128, 512], BF16, tag="vT")
                nc.vector.tensor_copy(vT[:, :], pv[:, :])
                for b in range(BPC):
                    pvn = ps_ab.tile([128, 4, 64], BF16, tag="ab")
                    bs = slice(b * 64, (b + 1) * 64)
                    for t4 in range(4):
                        nc.tensor.matmul(
                            pvn[:, t4, :],
                            vT[bs, t4 * 128:(t4 + 1) * 128],
                            ident[bs, bs], is_transpose=True)
                    nc.vector.tensor_copy(
                        vnt[b][:, blk * 4:(blk + 1) * 4, 0:64], pvn[:, :, :])

            def phase_c(bi):
                """Attention for T-row block bi (needs k/v blocks <= bi)."""
                oacc = ps_oa.tile([65, BPC, 512], F32, tag="oa")
                last = 4 * bi + 3

                def geom(j):
                    r = j - 4 * bi
                    w, c0 = (512, 0) if r <= 0 else (512 - 128 * r, 128 * r)
                    return r, w, c0

                def emit_qk(j):
                    r, w, c0 = geom(j)
                    js = slice(j * 128, (j + 1) * 128)
                    cs = slice(bi * 512 + c0, (bi + 1) * 512)
                    sab = ps_att.tile([128, BPC, 512], F32, tag="att")
                    nc.tensor.matmul(sab[:, 0, 0:w], skk[0:64, js],
                                     qk0[0:64, cs], start=True, stop=True,
                                     tile_position=(0, 0))
                    nc.tensor.matmul(sab[:, 1, 0:w], skk[64:128, js],
                                     qk1[64:128, cs], start=True, stop=True,
                                     tile_position=(64, 0))
                    return sab

                # software-pipelined: QK(j+1) is emitted before PV(j) so the
                # PE queue never has a PV (waiting on exp) ahead of ready QK
                sab = emit_qk(0)
                for j in range(last + 1):
                    r, w, c0 = geom(j)
                    pt = ptp.tile([128, BPC, 512], BF16, tag="pt")
                    nc.scalar.activation(pt[:, :, 0:w], sab[:, :, 0:w],
                                         mybir.ActivationFunctionType.Exp,
                                         scale=SCALE)
                    if r >= 0:
                        # keep where within-tile free idx >= partition idx.
                        # Since partitions only span 0..127, columns >= 128
                        # always pass: mask just the first 128 columns.
                        nc.gpsimd.affine_select(
                            out=pt[:, :, 0:128], in_=pt[:, :, 0:128],
                            compare_op=mybir.AluOpType.is_ge, fill=0.0,
                            base=0, pattern=[[0, BPC], [1, 128]],
                            channel_multiplier=-1)
                    if j < last:
                        sab = emit_qk(j + 1)
                    for b in range(BPC):
                        nc.tensor.matmul(oacc[:, b, c0:512], vnt[b][:, j, :],
                                         pt[:, b, 0:w], start=(j == 0),
                                         stop=(j == last))
                for b in range(BPC):
                    oex = oxp.tile([65, 512], F32, tag="oex")
                    # on the last block ACT is idle after its final exp; give
                    # it batch 1's drain so the two finalize chains overlap
                    if bi == NB - 1 and b == 1:
                        nc.scalar.copy(oex[:, :], oacc[:, b, :])
                    else:
                        nc.vector.tensor_copy(oex[:, :], oacc[:, b, :])
                    # NOTE: must NOT share the "ab" ring — that would make
                    # block k+1's transposes wait on this finalize
                    pso = ps_att.tile([128, 4, 65], F32, tag="att")
                    for t4 in range(4):
                        nc.tensor.matmul(pso[:, t4, :],
                                         oex[0:65, t4 * 128:(t4 + 1) * 128],
                                         identf[0:65, 0:65],
                                         is_transpose=True)
                    ot = fin.tile([128, 4, 65], F32, tag="ot")
                    if bi == NB - 1 and b == 1:
                        nc.scalar.copy(ot[:, :, :], pso[:, :, :])
                    else:
                        nc.vector.tensor_copy(ot[:, :, :], pso[:, :, :])
                    yt = fin.tile([128, 4, 64], F32, tag="yt")
                    if bi < NB - 1:
                        for t4 in range(4):
                            # out = ot / l on gpsimd; overwrites the l
                            # column with its reciprocal (unused afterwards)
                            nc.gpsimd.normalize_recip(yt[:, t4, :],
                                                      ot[:, t4, 0:64],
                                                      ot[:, t4, 64:65])
                    else:
                        # last block: Pool's serial ISA ops would sit on the
                        # kernel tail; DVE is idle there
                        linv = fin.tile([128, 4], F32, tag="linv")
                        nc.vector.reciprocal(linv[:, :], ot[:, :, 64])
                        for t4 in range(4):
                            nc.vector.tensor_scalar_mul(yt[:, t4, :],
                                                        ot[:, t4, 0:64],
                                                        linv[:, t4:t4 + 1])
                    nc.sync.dma_start(
                        out=y_d[b, bi * 512:(bi + 1) * 512, :].rearrange(
                            "(f p) h -> p f h", p=128),
                        in_=yt[:, :, :])

            for blk in range(NB):
                phase_ab(blk)
                phase_c(blk)

        if reps == 1:
            body()
        else:
            with tc.For_i(0, reps, 1):
                body()

    nc.compile()
    return nc


def build_program_v2(reps=1, att_bufs=2, ab_bufs=2, pt_bufs=3, x_bufs=3,
                     xt_bufs=2, qsplit=True, vsplit=True):
    """v2: natural-layout V (stationary=xT tile, moving=Wv) and swapped PV
    (stationary=scores tile, moving=v|ones) so attention output lands in
    natural [T, H] layout with the softmax denominator as a free 65th
    column -> no vn/finalize transposes. Scores stay bf16 row-tiled pairs
    (fp8 DoubleRow measured SLOWER than bf16 on HW despite the cost model's
    0.5 cyc/row: 333.8ns vs 213.4ns per row-tiled pair)."""
    from contextlib import ExitStack

    nc = bacc.Bacc("TRN2", target_bir_lowering=False, debug=False,
                   num_devices=NCORES)
    x_d = nc.dram_tensor("x", [BPC, T, C], F32, kind="ExternalInput").ap()
    wq_d = nc.dram_tensor("Wq", [C, H], F32, kind="ExternalInput").ap()
    wk_d = nc.dram_tensor("Wk", [C, H], F32, kind="ExternalInput").ap()
    wv_d = nc.dram_tensor("Wv", [C, H], F32, kind="ExternalInput").ap()
    y_d = nc.dram_tensor("y", [BPC, T, H], F32, kind="ExternalOutput").ap()

    with tile.TileContext(nc) as tc, ExitStack() as ctx:
        singles = ctx.enter_context(tc.tile_pool(name="singles", bufs=1))
        xp = ctx.enter_context(tc.tile_pool(name="xp", bufs=x_bufs))
        xTp = ctx.enter_context(tc.tile_pool(name="xTp", bufs=xt_bufs))
        qkp = ctx.enter_context(tc.tile_pool(name="qkp", bufs=1))
        vnp = ctx.enter_context(tc.tile_pool(name="vnp", bufs=1))
        ptp = ctx.enter_context(tc.tile_pool(name="ptp", bufs=pt_bufs))
        fin = ctx.enter_context(tc.tile_pool(name="fin", bufs=2))
        ps_att = ctx.enter_context(tc.tile_pool(name="psatt", bufs=att_bufs,
                                                space="PSUM"))
        ps_oa = ctx.enter_context(tc.tile_pool(name="psoa", bufs=1,
                                               space="PSUM"))
        ps_ab = ctx.enter_context(tc.tile_pool(name="psab", bufs=ab_bufs,
                                               space="PSUM"))

        with tc.high_priority():
            ident = singles.tile([128, 128], BF16)
            make_identity(nc, ident[:, :])
            # PE warmup during the first x transfer
            warm = ps_att.tile([128, BPC, 512], F32, tag="att")
            for i in range(6):
                nc.tensor.matmul(warm[:, i % 2, 0:128], ident[:, :],
                                 ident[:, :], start=True, stop=True)

        # fp32 weight staging via HWDGE, pack on Pool. b0 packs [Wq|Wk],
        # b1 [Wk|Wq] so q lands on the batch's own row half with a plain
        # copy (same as v1).
        wq_s = singles.tile([128, CB, 64], F32)
        wk_s = singles.tile([128, CB, 64], F32)
        wv_s = singles.tile([128, CB, 64], F32)
        nc.sync.dma_start(out=wq_s[:, :, :],
                          in_=wq_d.rearrange("(c p) h -> p c h", p=128))
        nc.sync.dma_start(out=wk_s[:, :, :],
                          in_=wk_d.rearrange("(c p) h -> p c h", p=128))
        nc.sync.dma_start(out=wv_s[:, :, :],
                          in_=wv_d.rearrange("(c p) h -> p c h", p=128))
        wqk0 = singles.tile([128, CB, 128], BF16)
        nc.gpsimd.tensor_copy(wqk0[:, :, 0:64], wq_s[:, :, :])
        nc.gpsimd.tensor_copy(wqk0[:, :, 64:128], wk_s[:, :, :])
        wqk1 = singles.tile([128, CB, 128], BF16)
        nc.gpsimd.tensor_copy(wqk1[:, :, 0:64], wk_s[:, :, :])
        nc.gpsimd.tensor_copy(wqk1[:, :, 64:128], wq_s[:, :, :])
        wqk = (wqk0, wqk1)
        wv_b = singles.tile([128, CB, 64], BF16)
        nc.gpsimd.tensor_copy(wv_b[:, :, :], wv_s[:, :, :])

        def body():
            # q^T for both batches: qk0 rows 0:64 = q0, qk1 rows 64:128 = q1
            qk0 = qkp.tile([128, T], BF16, tag="qk0")
            qk1 = qkp.tile([128, T], BF16, tag="qk1")
            # k^T for both batches: rows 0:64 = k0, rows 64:128 = k1
            skk = qkp.tile([128, T], BF16, tag="skk")
            vn0 = vnp.tile([128, TT, 65], BF16, tag="vn0")
            vn1 = vnp.tile([128, TT, 65], BF16, tag="vn1")
            nc.vector.memset(vn0[:, :, 64], 1.0)
            nc.vector.memset(vn1[:, :, 64], 1.0)
            qkt = (qk0, qk1)
            vnt = (vn0, vn1)

            def phase_ab(blk):
                xTs = []
                for b in range(BPC):
                    xT = xTp.tile([128, CB, 512], BF16, tag=f"xT{b}")
                    xTs.append(xT)
                    xt = xp.tile([128, 4, C], BF16, tag="x")
                    base = blk * 512
                    if blk == 0 and b == 0:
                        with tc.high_priority(offset=1 << 20):
                            nc.gpsimd.dma_start(
                                out=xt[:, 0:2, :],
                                in_=x_d[b, base:base + 256, :].rearrange(
                                    "(f p) c -> p f c", p=128))
                        with tc.high_priority():
                            nc.gpsimd.dma_start(
                                out=xt[:, 2:4, :],
                                in_=x_d[b, base + 256:base + 512, :].rearrange(
                                    "(f p) c -> p f c", p=128))
                    else:
                        nc.gpsimd.dma_start(
                            out=xt[:, :, :],
                            in_=x_d[b, base:base + 512, :].rearrange(
                                "(f p) c -> p f c", p=128))
                    for t4 in range(4):
                        ptr = ps_ab.tile([128, CB, 128], BF16, tag="ab")
                        for ci in range(CB):
                            nc.tensor.matmul(ptr[:, ci, :],
                                             xt[:, t4, ci * 128:(ci + 1) * 128],
                                             ident[:, :], is_transpose=True)
                        nc.vector.tensor_copy(
                            xT[:, :, t4 * 128:(t4 + 1) * 128], ptr[:, :, :])
                sl = slice(blk * 512, (blk + 1) * 512)
                # Projections with split-K: each 128-row chunk becomes lo/hi
                # 64-row matmuls at tile positions (0,0)/(64,0) accumulating
                # into two separate PSUM banks (measured 2.2x the K=128
                # stream rate); a DVE add fuses the partials straight to
                # bf16. v partials borrow the oa banks (idle during AB) so
                # the ab ring never waits on the adds.
                for b in range(BPC):
                    if qsplit:
                        pqA = ps_ab.tile([128, 512], F32, tag="ab")
                        pqB = ps_ab.tile([128, 512], F32, tag="ab")
                        for ci in range(CB):
                            nc.tensor.matmul(pqA[:, :], wqk[b][0:64, ci, :],
                                             xTs[b][0:64, ci, :],
                                             start=(ci == 0),
                                             stop=(ci == CB - 1),
                                             tile_position=(0, 0))
                            nc.tensor.matmul(pqB[:, :], wqk[b][64:128, ci, :],
                                             xTs[b][64:128, ci, :],
                                             start=(ci == 0),
                                             stop=(ci == CB - 1),
                                             tile_position=(64, 0))
                        # walrus rejects dual-PSUM-input DVE ops: stage A
                        # to the SBUF destination, then add B in place
                        nc.vector.tensor_copy(qkt[b][:, sl], pqA[:, :])
                        nc.vector.scalar_tensor_tensor(
                            out=qkt[b][:, sl], in0=qkt[b][:, sl], scalar=0.0,
                            in1=pqB[:, :], op0=mybir.AluOpType.add,
                            op1=mybir.AluOpType.add)
                    else:
                        pq = ps_ab.tile([128, 512], F32, tag="ab")
                        for ci in range(CB):
                            nc.tensor.matmul(pq[:, :], wqk[b][:, ci, :],
                                             xTs[b][:, ci, :],
                                             start=(ci == 0),
                                             stop=(ci == CB - 1))
                        nc.vector.tensor_copy(qkt[b][:, sl], pq[:, :])
                    if vsplit:
                        # v in natural [t, h]: same split, partials in the
                        # oa banks (idle during AB)
                        pvvA = ps_oa.tile([128, 4, 64], F32, tag="oa0")
                        pvvB = ps_oa.tile([128, 4, 64], F32, tag="oa1")
                        for t4 in range(4):
                            ts = slice(t4 * 128, (t4 + 1) * 128)
                            for ci in range(CB):
                                nc.tensor.matmul(pvvA[:, t4, :],
                                                 xTs[b][0:64, ci, ts],
                                                 wv_b[0:64, ci, :],
                                                 start=(ci == 0),
                                                 stop=(ci == CB - 1),
                                                 tile_position=(0, 0),
                                                 skip_group_check=True)
                                nc.tensor.matmul(pvvB[:, t4, :],
                                                 xTs[b][64:128, ci, ts],
                                                 wv_b[64:128, ci, :],
                                                 start=(ci == 0),
                                                 stop=(ci == CB - 1),
                                                 tile_position=(64, 0),
                                                 skip_group_check=True)
                        vsl = vnt[b][:, blk * 4:(blk + 1) * 4, 0:64]
                        nc.vector.tensor_copy(vsl, pvvA[:, :, :])
                        nc.vector.scalar_tensor_tensor(
                            out=vsl, in0=vsl, scalar=0.0,
                            in1=pvvB[:, :, :], op0=mybir.AluOpType.add,
                            op1=mybir.AluOpType.add)
                    else:
                        pvv = ps_ab.tile([128, 4, 64], F32, tag="ab")
                        for t4 in range(4):
                            ts = slice(t4 * 128, (t4 + 1) * 128)
                            for ci in range(CB):
                                nc.tensor.matmul(pvv[:, t4, :],
                                                 xTs[b][:, ci, ts],
                                                 wv_b[:, ci, :],
                                                 start=(ci == 0),
                                                 stop=(ci == CB - 1))
                        nc.vector.tensor_copy(
                            vnt[b][:, blk * 4:(blk + 1) * 4, 0:64],
                            pvv[:, :, :])
                # k partition shifts into the shared k tile
                nc.sync.dma_start(out=skk[0:64, sl], in_=qk0[64:128, sl])
                nc.sync.dma_start(out=skk[64:128, sl], in_=qk1[0:64, sl])

            def phase_c(bi):
                # one full 2KB bank per batch: a single bank-wide PSUM
                # accumulation group (start on the very first matmul into the
                # bank, stop on the last). Regions zero lazily on first
                # touch; interleaved per-region start/stops would corrupt
                # neighbours (pending-zero arms the whole 2KB zero region).
                oacc0 = ps_oa.tile([128, 4, 128], F32, tag="oa0")
                oacc1 = ps_oa.tile([128, 4, 128], F32, tag="oa1")
                oaccs = (oacc0, oacc1)
                last = 4 * bi + 3

                def geom(j):
                    r = j - 4 * bi
                    w, c0 = (512, 0) if r <= 0 else (512 - 128 * r, 128 * r)
                    return r, w, c0

                def emit_qk(j):
                    r, w, c0 = geom(j)
                    js = slice(j * 128, (j + 1) * 128)
                    cs = slice(bi * 512 + c0, (bi + 1) * 512)
                    sab = ps_att.tile([128, BPC, 512], F32, tag="att")
                    nc.tensor.matmul(sab[:, 0, 0:w], skk[0:64, js],
                                     qk0[0:64, cs], start=True, stop=True,
                                     tile_position=(0, 0))
                    nc.tensor.matmul(sab[:, 1, 0:w], skk[64:128, js],
                                     qk1[64:128, cs], start=True, stop=True,
                                     tile_position=(64, 0))
                    return sab

                sab = emit_qk(0)
                for j in range(last + 1):
                    r, w, c0 = geom(j)
                    pt = ptp.tile([128, BPC, 512], BF16, tag="pt")
                    nc.scalar.activation(pt[:, :, 0:w], sab[:, :, 0:w],
                                         mybir.ActivationFunctionType.Exp,
                                         scale=SCALE)
                    if r >= 0:
                        nc.gpsimd.affine_select(
                            out=pt[:, :, 0:128], in_=pt[:, :, 0:128],
                            compare_op=mybir.AluOpType.is_ge, fill=0.0,
                            base=0, pattern=[[0, BPC], [1, 128]],
                            channel_multiplier=-1)
                    if j < last:
                        sab = emit_qk(j + 1)
                    # PV: stationary = scores tile (s x t), moving = v|ones
                    for tau in range(max(r, 0), 4):
                        p0 = tau * 128 - c0
                        for b in range(BPC):
                            nc.tensor.matmul(oaccs[b][:, tau, 0:65],
                                             pt[:, b, p0:p0 + 128],
                                             vnt[b][:, j, :],
                                             start=(j == 0 and tau == max(r, 0)),
                                             stop=(j == last and tau == 3),
                                             skip_group_check=True)
                for b in range(BPC):
                    ot = fin.tile([128, 4, 65], F32, tag="ot")
                    if bi == NB - 1 and b == 1:
                        nc.scalar.copy(ot[:, :, :], oaccs[b][:, :, 0:65])
                    else:
                        nc.vector.tensor_copy(ot[:, :, :], oaccs[b][:, :, 0:65])
                    yt = fin.tile([128, 4, 64], F32, tag="yt")
                    if bi < NB - 1:
                        for t4 in range(4):
                            nc.gpsimd.normalize_recip(yt[:, t4, :],
                                                      ot[:, t4, 0:64],
                                                      ot[:, t4, 64:65])
                    else:
                        linv = fin.tile([128, 4], F32, tag="linv")
                        nc.vector.reciprocal(linv[:, :], ot[:, :, 64])
                        for t4 in range(4):
                            nc.vector.tensor_scalar_mul(yt[:, t4, :],
                                                        ot[:, t4, 0:64],
                                                        linv[:, t4:t4 + 1])
                    nc.sync.dma_start(
                        out=y_d[b, bi * 512:(bi + 1) * 512, :].rearrange(
                            "(f p) h -> p f h", p=128),
                        in_=yt[:, :, :])

            for blk in range(NB):
                phase_ab(blk)
                phase_c(blk)

        if reps == 1:
            body()
        else:
            with tc.For_i(0, reps, 1):
                body()

    nc.compile()
    return nc


_CACHE = {}


def _get_program(**kw):
    key = tuple(sorted(kw.items()))
    if key not in _CACHE:
        kw2 = dict(kw)
        v = kw2.pop("v", 2)
        fn = build_program_v2 if int(v) == 2 else build_program
        _CACHE[key] = fn(**kw2)
    return _CACHE[key]


def run_sharded(x, Wq, Wk, Wv, trace=False, **build_kw):
    """Run on 8 cores, return (y_full, BassKernelResults)."""
    nc = _get_program(**build_kw)
    x = np.ascontiguousarray(np.asarray(x, dtype=np.float32))
    Wq = np.ascontiguousarray(np.asarray(Wq, dtype=np.float32))
    Wk = np.ascontiguousarray(np.asarray(Wk, dtype=np.float32))
    Wv = np.ascontiguousarray(np.asarray(Wv, dtype=np.float32))
    xs = x.reshape(NCORES, BPC, T, C)
    in_maps = [{"x": np.ascontiguousarray(xs[i]), "Wq": Wq, "Wk": Wk, "Wv": Wv}
               for i in range(NCORES)]
    res = run_bass_kernel_spmd(nc, in_maps, list(range(NCORES)), trace=trace)
    y = np.stack([res.results[i]["y"] for i in range(NCORES)], axis=0)
    return y.reshape(B, T, H), res


def kernel(x, Wq, Wk, Wv):
    y, _ = run_sharded(x, Wq, Wk, Wv, trace=False)
    return y


# ---------------- timing support (no NTFF profiler in this container) ----


def make_runner(nc, n_iter=1):
    """Build a reusable sharded jit callable for `nc` (mirrors
    bass2jax.run_bass_via_pjrt's multi-core path, without donation so
    device inputs can be reused across timed calls)."""
    import jax
    from jax.sharding import Mesh, PartitionSpec
    try:
        from jax.experimental.shard_map import shard_map
    except ImportError:  # newer jax
        from jax.shard_map import shard_map
    from concourse import bass2jax
    bass2jax.install_neuronx_cc_hook()

    part_name = (nc.partition_id_tensor.name if nc.partition_id_tensor
                 else None)
    in_names, out_names, out_avals, zero_outs = [], [], [], []
    for alloc in nc.m.functions[0].allocations:
        if not isinstance(alloc, mybir.MemoryLocationSet):
            continue
        name = alloc.memorylocations[0].name
        if alloc.kind == "ExternalInput":
            if name != part_name:
                in_names.append(name)
        elif alloc.kind == "ExternalOutput":
            out_names.append(name)
            shape = tuple(alloc.tensor_shape)
            dtype = mybir.dt.np(alloc.dtype)
            out_avals.append(jax.core.ShapedArray(shape, dtype))
            zero_outs.append(np.zeros(shape, dtype))
    n_params = len(in_names)
    all_names = in_names + out_names
    if part_name is not None:
        all_names = all_names + [part_name]

    def _body(*args):
        ins = list(args[:n_params])
        youts = list(args[n_params:n_params + len(out_names)])
        for _ in range(n_iter):
            operands = ins + youts
            if part_name is not None:
                operands.append(bass2jax.partition_id_tensor())
            outs = bass2jax._bass_exec_p.bind(
                *operands, out_avals=tuple(out_avals),
                in_names=tuple(all_names), out_names=tuple(out_names),
                lowering_input_output_aliases=(),
                sim_require_finite=True, sim_require_nnan=True, nc=nc)
            youts = list(outs)
        return tuple(youts)

    devices = jax.devices()[:NCORES]
    mesh = Mesh(np.asarray(devices), ("core",))
    in_specs = (PartitionSpec("core"),) * (n_params + len(out_names))
    out_specs = (PartitionSpec("core"),) * len(out_names)
    fn = jax.jit(shard_map(_body, mesh=mesh, in_specs=in_specs,
                           out_specs=out_specs, check_rep=False),
                 keep_unused=True)
    return fn, in_names, zero_outs, mesh


def _timed_calls(fn, dev_in, iters):
    import time as _time
    import jax
    out = fn(*dev_in)
    jax.block_until_ready(out)
    ts = []
    for _ in range(iters):
        t0 = _time.perf_counter_ns()
        out = fn(*dev_in)
        jax.block_until_ready(out)
        ts.append(_time.perf_counter_ns() - t0)
    ts.sort()
    return ts


def time_calls(nc, in_maps, iters=10):
    """Sorted wall times (ns) of warm sharded calls of nc's NEFF."""
    import jax
    from jax.sharding import NamedSharding, PartitionSpec
    fn, in_names, zero_outs, mesh = make_runner(nc, n_iter=1)
    sh = NamedSharding(mesh, PartitionSpec("core"))
    concat = [np.concatenate([np.asarray(m[n]) for m in in_maps], axis=0)
              for n in in_names]
    concat += [np.zeros((NCORES * z.shape[0], *z.shape[1:]), z.dtype)
               for z in zero_outs]
    dev_in = [jax.device_put(a, sh) for a in concat]
    return _timed_calls(fn, dev_in, iters)


_BASELINE = {}


def baseline_nc():
    """Tiny kernel to measure the axon dispatch floor."""
    if "nc" in _BASELINE:
        return _BASELINE["nc"]
    nc = bacc.Bacc("TRN2", target_bir_lowering=False, debug=False,
                   num_devices=NCORES)
    a = nc.dram_tensor("a", [128, 128], F32, kind="ExternalInput").ap()
    b = nc.dram_tensor("b", [128, 128], F32, kind="ExternalOutput").ap()
    with tile.TileContext(nc) as tc:
        with tc.tile_pool(name="p", bufs=1) as pool:
            t = pool.tile([128, 128], F32)
            nc.sync.dma_start(out=t[:, :], in_=a)
            nc.sync.dma_start(out=b, in_=t[:, :])
    nc.compile()
    _BASELINE["nc"] = nc
    return nc



# revision 12
# speedup vs baseline: 1.0557x; 1.0557x over previous
"""Single-head causal attention on 8 trn2 NeuronCores.

Problem: x[16, 2048, 1024] fp32, Wq/Wk/Wv[1024, 64] fp32 ->
         out[16, 2048, 64] = softmax(causal(q k^T / sqrt(64))) v

Sharding: data-parallel over batch B=16 -> 2 batches per core, no
collectives. SPMD program; the two per-core batches are processed in
lockstep so small-dim matmuls can be packed across them.

Key structure (vs the fp32/f32r baseline this replaces):
  * x is loaded with gpsimd casting DMAs (fp32 HBM -> bf16 SBUF,
    round-to-nearest-even, verified on HW). Everything downstream of
    the load runs in bf16 except PSUM accumulation (always fp32) and
    the final normalize, so PE transposes run at 1 cyc/row and DVE
    copies at the 2-byte 2x rate. Max rel err vs fp64 reference is
    ~4e-3 (gate 2e-2).
  * x^T tiles via PE transpose (8 chunks share one bf16 PSUM bank,
    one wide DVE copy out).
  * Projections: per-batch packed [Wq|Wk] (b0) / [Wk|Wq] (b1) so that
    q0 lands at partitions 0:64 and q1 at 64:128 with plain copies;
    k halves are partition-shifted into a shared k tile by SBUF->SBUF
    DMA. v for both batches in one pass via column tiling
    (tile_position (0,0)/(0,64)), then PE-transposed to natural [T,64]
    with a ones column appended (PV then emits the softmax denominator
    for free).
  * Scores S^T: the two batches' K=64 matmuls run concurrently in the
    PE array via row tiling (tile_position (0,0)/(64,0)) into the two
    halves of one [128,2,512] PSUM pair; exp and causal mask are one
    instruction per j for both batches.
  * Schedule: AB(blk) then C(blk) per T-block, so block k+1's loads/
    transposes overlap block k's attention. The attention j-loop is
    software-pipelined (QK for j+1 is emitted before PV(j)) so the PE
    queue never parks a PV (waiting on exp) ahead of ready score work.
    PSUM is exactly allocated: 4 banks attention ring, 2 banks output
    accumulators, 2 banks load/transpose ring - the finalize transposes
    deliberately use the attention ring, NOT the "ab" ring, so block
    k+1's transposes never wait on block k's finalize.

Cost-model timeline (sim.py): 104.6 us. HW per-rep slope: ~110-118 us
(environment-dependent). Causal mask applies only to the first 128
columns of diagonal tiles (columns >= 128 pass vacuously).
Remaining gap over engine-busy: ~8 us DMA-bound startup, ~5 us serial
finalize tail, ~17 us of per-dependency semaphore latency, and the
exp-volume floor on ACT (~29 us) pacing the causally-last row block.
"""

import sys

sys.path.insert(0, "/opt/trn_rl_repo")

import numpy as np

import concourse.bass as bass  # noqa: F401
import concourse.bacc as bacc
import concourse.mybir as mybir
import concourse.tile as tile
from concourse.masks import make_identity
from concourse.bass_utils import run_bass_kernel_spmd

B, T, C, H = 16, 2048, 1024, 64
NCORES = 8
BPC = B // NCORES  # batches per core
CB = C // 128      # 8 contraction chunks
TT = T // 128      # 16 T tiles of 128
NB = T // 512      # 4 T blocks of 512
F32 = mybir.dt.float32
BF16 = mybir.dt.bfloat16
SCALE = float(H) ** -0.5


def build_program(reps=1, att_bufs=2, ab_bufs=2, pt_bufs=3, x_bufs=3,
                  xt_bufs=2, oex_bufs=2):
    from contextlib import ExitStack

    nc = bacc.Bacc("TRN2", target_bir_lowering=False, debug=False,
                   num_devices=NCORES)
    x_d = nc.dram_tensor("x", [BPC, T, C], F32, kind="ExternalInput").ap()
    wq_d = nc.dram_tensor("Wq", [C, H], F32, kind="ExternalInput").ap()
    wk_d = nc.dram_tensor("Wk", [C, H], F32, kind="ExternalInput").ap()
    wv_d = nc.dram_tensor("Wv", [C, H], F32, kind="ExternalInput").ap()
    y_d = nc.dram_tensor("y", [BPC, T, H], F32, kind="ExternalOutput").ap()

    with tile.TileContext(nc) as tc, ExitStack() as ctx:
        singles = ctx.enter_context(tc.tile_pool(name="singles", bufs=1))
        xp = ctx.enter_context(tc.tile_pool(name="xp", bufs=x_bufs))
        xTp = ctx.enter_context(tc.tile_pool(name="xTp", bufs=xt_bufs))
        qkp = ctx.enter_context(tc.tile_pool(name="qkp", bufs=1))
        vnp = ctx.enter_context(tc.tile_pool(name="vnp", bufs=1))
        vTp = ctx.enter_context(tc.tile_pool(name="vTp", bufs=2))
        ptp = ctx.enter_context(tc.tile_pool(name="ptp", bufs=pt_bufs))
        oxp = ctx.enter_context(tc.tile_pool(name="oxp", bufs=oex_bufs))
        fin = ctx.enter_context(tc.tile_pool(name="fin", bufs=2))
        ps_att = ctx.enter_context(tc.tile_pool(name="psatt", bufs=att_bufs,
                                                space="PSUM"))
        ps_oa = ctx.enter_context(tc.tile_pool(name="psoa", bufs=1,
                                               space="PSUM"))
        ps_ab = ctx.enter_context(tc.tile_pool(name="psab", bufs=ab_bufs,
                                               space="PSUM"))

        # identity build is dependency-free Pool work; run it before the x
        # descriptor generation so the first transposes aren't blocked on it
        with tc.high_priority():
            ident = singles.tile([128, 128], BF16)
            make_identity(nc, ident[:, :])
            identf = singles.tile([128, 128], F32)
            make_identity(nc, identf[:, :])
            # warm the PE array (HAM clock ramp) with dummy matmuls while
            # the first x transfer is in flight; they borrow an attention
            # ring slot that isn't needed until ~13us in
            warm = ps_att.tile([128, BPC, 512], F32, tag="att")
            for i in range(24):
                nc.tensor.matmul(warm[:, i % 2, 0:128], ident[:, :],
                                 ident[:, :], start=True, stop=True)

        # Weights: fp32 staging via HWDGE (keeps Pool free for the x casting
        # DMAs at startup), then convert/pack on gpsimd. b0 packs [Wq|Wk],
        # b1 [Wk|Wq] so q lands on the batch's own row half with a plain
        # copy.
        wq_s = singles.tile([128, CB, 64], F32)
        wk_s = singles.tile([128, CB, 64], F32)
        wv_s = singles.tile([128, CB, 64], F32)
        nc.sync.dma_start(out=wq_s[:, :, :],
                          in_=wq_d.rearrange("(c p) h -> p c h", p=128))
        nc.sync.dma_start(out=wk_s[:, :, :],
                          in_=wk_d.rearrange("(c p) h -> p c h", p=128))
        nc.sync.dma_start(out=wv_s[:, :, :],
                          in_=wv_d.rearrange("(c p) h -> p c h", p=128))
        # packing on Pool: these wait on the weight transfers, and on DVE
        # they would head-of-line block the first transpose copies
        wqk0 = singles.tile([128, CB, 128], BF16)
        nc.gpsimd.tensor_copy(wqk0[:, :, 0:64], wq_s[:, :, :])
        nc.gpsimd.tensor_copy(wqk0[:, :, 64:128], wk_s[:, :, :])
        wqk1 = singles.tile([128, CB, 128], BF16)
        nc.gpsimd.tensor_copy(wqk1[:, :, 0:64], wk_s[:, :, :])
        nc.gpsimd.tensor_copy(wqk1[:, :, 64:128], wq_s[:, :, :])
        wv2 = singles.tile([128, CB, 128], BF16)
        nc.gpsimd.tensor_copy(wv2[:, :, 0:64], wv_s[:, :, :])
        nc.gpsimd.tensor_copy(wv2[:, :, 64:128], wv_s[:, :, :])
        wqk = (wqk0, wqk1)

        def body():
            # q^T for both batches: qk0 rows 0:64 = q0, qk1 rows 64:128 = q1
            qk0 = qkp.tile([128, T], BF16, tag="qk0")
            qk1 = qkp.tile([128, T], BF16, tag="qk1")
            # k^T for both batches: rows 0:64 = k0, rows 64:128 = k1
            skk = qkp.tile([128, T], BF16, tag="skk")
            vn0 = vnp.tile([128, TT, 65], BF16, tag="vn0")
            vn1 = vnp.tile([128, TT, 65], BF16, tag="vn1")
            nc.vector.memset(vn0[:, :, 64], 1.0)
            nc.vector.memset(vn1[:, :, 64], 1.0)
            qkt = (qk0, qk1)
            vnt = (vn0, vn1)

            def phase_ab(blk):
                """Load + transpose x, projections for T block blk."""
                xTs = []
                for b in range(BPC):
                    xT = xTp.tile([128, CB, 512], BF16, tag=f"xT{b}")
                    xTs.append(xT)
                    # one casting DMA per (batch, block): 1 MB fp32 -> bf16.
                    # The very first block is split in two so the transpose
                    # pipeline fills sooner.
                    xt = xp.tile([128, 4, C], BF16, tag="x")
                    base = blk * 512
                    if blk == 0 and b == 0:
                        # first load split in two and boosted ahead of
                        # everything (even the identity build: it hides
                        # under the transfer) so the pipeline fills asap
                        with tc.high_priority(offset=1 << 20):
                            nc.gpsimd.dma_start(
                                out=xt[:, 0:2, :],
                                in_=x_d[b, base:base + 256, :].rearrange(
                                    "(f p) c -> p f c", p=128))
                        with tc.high_priority():
                            nc.gpsimd.dma_start(
                                out=xt[:, 2:4, :],
                                in_=x_d[b, base + 256:base + 512, :].rearrange(
                                    "(f p) c -> p f c", p=128))
                    else:
                        nc.gpsimd.dma_start(
                            out=xt[:, :, :],
                            in_=x_d[b, base:base + 512, :].rearrange(
                                "(f p) c -> p f c", p=128))
                    for t4 in range(4):
                        ptr = ps_ab.tile([128, CB, 128], BF16, tag="ab")
                        for ci in range(CB):
                            nc.tensor.matmul(ptr[:, ci, :],
                                             xt[:, t4, ci * 128:(ci + 1) * 128],
                                             ident[:, :], is_transpose=True)
                        nc.vector.tensor_copy(
                            xT[:, :, t4 * 128:(t4 + 1) * 128], ptr[:, :, :])
                sl = slice(blk * 512, (blk + 1) * 512)
                for b in range(BPC):
                    pq = ps_ab.tile([128, 512], F32, tag="ab")
                    for ci in range(CB):
                        nc.tensor.matmul(pq[:, :], wqk[b][:, ci, :],
                                         xTs[b][:, ci, :],
                                         start=(ci == 0), stop=(ci == CB - 1))
                    nc.vector.tensor_copy(qkt[b][:, sl], pq[:, :])
                # k partition shifts into the shared k tile
                nc.sync.dma_start(out=skk[0:64, sl], in_=qk0[64:128, sl])
                nc.sync.dma_start(out=skk[64:128, sl], in_=qk1[0:64, sl])
                # v for both batches, column-tiled into one PSUM bank
                pv = ps_ab.tile([128, 512], F32, tag="ab")
                for ci in range(CB):
                    nc.tensor.matmul(pv[0:64, :], wv2[:, ci, 0:64],
                                     xTs[0][:, ci, :], start=(ci == 0),
                                     stop=(ci == CB - 1), tile_position=(0, 0))
                    nc.tensor.matmul(pv[64:128, :], wv2[:, ci, 64:128],
                                     xTs[1][:, ci, :], start=(ci == 0),
                                     stop=(ci == CB - 1), tile_position=(0, 64))
                vT = vTp.tile([128, 512], BF16, tag="vT")
                nc.vector.tensor_copy(vT[:, :], pv[:, :])
                for b in range(BPC):
                    pvn = ps_ab.tile([128, 4, 64], BF16, tag="ab")
                    bs = slice(b * 64, (b + 1) * 64)
                    for t4 in range(4):
                        nc.tensor.matmul(
                            pvn[:, t4, :],
                            vT[bs, t4 * 128:(t4 + 1) * 128],
                            ident[bs, bs], is_transpose=True)
                    nc.vector.tensor_copy(
                        vnt[b][:, blk * 4:(blk + 1) * 4, 0:64], pvn[:, :, :])

            def phase_c(bi):
                """Attention for T-row block bi (needs k/v blocks <= bi)."""
                oacc = ps_oa.tile([65, BPC, 512], F32, tag="oa")
                last = 4 * bi + 3

                def geom(j):
                    r = j - 4 * bi
                    w, c0 = (512, 0) if r <= 0 else (512 - 128 * r, 128 * r)
                    return r, w, c0

                def emit_qk(j):
                    r, w, c0 = geom(j)
                    js = slice(j * 128, (j + 1) * 128)
                    cs = slice(bi * 512 + c0, (bi + 1) * 512)
                    sab = ps_att.tile([128, BPC, 512], F32, tag="att")
                    nc.tensor.matmul(sab[:, 0, 0:w], skk[0:64, js],
                                     qk0[0:64, cs], start=True, stop=True,
                                     tile_position=(0, 0))
                    nc.tensor.matmul(sab[:, 1, 0:w], skk[64:128, js],
                                     qk1[64:128, cs], start=True, stop=True,
                                     tile_position=(64, 0))
                    return sab

                # software-pipelined: QK(j+1) is emitted before PV(j) so the
                # PE queue never has a PV (waiting on exp) ahead of ready QK
                sab = emit_qk(0)
                for j in range(last + 1):
                    r, w, c0 = geom(j)
                    pt = ptp.tile([128, BPC, 512], BF16, tag="pt")
                    nc.scalar.activation(pt[:, :, 0:w], sab[:, :, 0:w],
                                         mybir.ActivationFunctionType.Exp,
                                         scale=SCALE)
                    if r >= 0:
                        # keep where within-tile free idx >= partition idx.
                        # Since partitions only span 0..127, columns >= 128
                        # always pass: mask just the first 128 columns.
                        nc.gpsimd.affine_select(
                            out=pt[:, :, 0:128], in_=pt[:, :, 0:128],
                            compare_op=mybir.AluOpType.is_ge, fill=0.0,
                            base=0, pattern=[[0, BPC], [1, 128]],
                            channel_multiplier=-1)
                    if j < last:
                        sab = emit_qk(j + 1)
                    for b in range(BPC):
                        nc.tensor.matmul(oacc[:, b, c0:512], vnt[b][:, j, :],
                                         pt[:, b, 0:w], start=(j == 0),
                                         stop=(j == last))
                for b in range(BPC):
                    oex = oxp.tile([65, 512], F32, tag="oex")
                    # on the last block ACT is idle after its final exp; give
                    # it batch 1's drain so the two finalize chains overlap
                    if bi == NB - 1 and b == 1:
                        nc.scalar.copy(oex[:, :], oacc[:, b, :])
                    else:
                        nc.vector.tensor_copy(oex[:, :], oacc[:, b, :])
                    # NOTE: must NOT share the "ab" ring — that would make
                    # block k+1's transposes wait on this finalize
                    pso = ps_att.tile([128, 4, 65], F32, tag="att")
                    for t4 in range(4):
                        nc.tensor.matmul(pso[:, t4, :],
                                         oex[0:65, t4 * 128:(t4 + 1) * 128],
                                         identf[0:65, 0:65],
                                         is_transpose=True)
                    ot = fin.tile([128, 4, 65], F32, tag="ot")
                    if bi == NB - 1 and b == 1:
                        nc.scalar.copy(ot[:, :, :], pso[:, :, :])
                    else:
                        nc.vector.tensor_copy(ot[:, :, :], pso[:, :, :])
                    yt = fin.tile([128, 4, 64], F32, tag="yt")
                    if bi < NB - 1:
                        for t4 in range(4):
                            # out = ot / l on gpsimd; overwrites the l
                            # column with its reciprocal (unused afterwards)
                            nc.gpsimd.normalize_recip(yt[:, t4, :],
                                                      ot[:, t4, 0:64],
                                                      ot[:, t4, 64:65])
                    else:
                        # last block: Pool's serial ISA ops would sit on the
                        # kernel tail; DVE is idle there
                        linv = fin.tile([128, 4], F32, tag="linv")
                        nc.vector.reciprocal(linv[:, :], ot[:, :, 64])
                        for t4 in range(4):
                            nc.vector.tensor_scalar_mul(yt[:, t4, :],
                                                        ot[:, t4, 0:64],
                                                        linv[:, t4:t4 + 1])
                    nc.sync.dma_start(
                        out=y_d[b, bi * 512:(bi + 1) * 512, :].rearrange(
                            "(f p) h -> p f h", p=128),
                        in_=yt[:, :, :])

            for blk in range(NB):
                phase_ab(blk)
                phase_c(blk)

        if reps == 1:
            body()
        else:
            with tc.For_i(0, reps, 1):
                body()

    nc.compile()
    return nc


def build_program_v2(reps=1, att_bufs=2, ab_bufs=2, pt_bufs=3, x_bufs=3,
                     xt_bufs=2, qsplit=False, vsplit=False):
    """v2: natural-layout V (stationary=xT tile, moving=Wv) and swapped PV
    (stationary=scores tile, moving=v|ones) so attention output lands in
    natural [T, H] layout with the softmax denominator as a free 65th
    column -> no vn/finalize transposes. Scores stay bf16 row-tiled pairs
    (fp8 DoubleRow measured SLOWER than bf16 on HW despite the cost model's
    0.5 cyc/row: 333.8ns vs 213.4ns per row-tiled pair)."""
    from contextlib import ExitStack

    nc = bacc.Bacc("TRN2", target_bir_lowering=False, debug=False,
                   num_devices=NCORES)
    x_d = nc.dram_tensor("x", [BPC, T, C], F32, kind="ExternalInput").ap()
    wq_d = nc.dram_tensor("Wq", [C, H], F32, kind="ExternalInput").ap()
    wk_d = nc.dram_tensor("Wk", [C, H], F32, kind="ExternalInput").ap()
    wv_d = nc.dram_tensor("Wv", [C, H], F32, kind="ExternalInput").ap()
    y_d = nc.dram_tensor("y", [BPC, T, H], F32, kind="ExternalOutput").ap()

    with tile.TileContext(nc) as tc, ExitStack() as ctx:
        singles = ctx.enter_context(tc.tile_pool(name="singles", bufs=1))
        xp = ctx.enter_context(tc.tile_pool(name="xp", bufs=x_bufs))
        xTp = ctx.enter_context(tc.tile_pool(name="xTp", bufs=xt_bufs))
        qkp = ctx.enter_context(tc.tile_pool(name="qkp", bufs=1))
        vnp = ctx.enter_context(tc.tile_pool(name="vnp", bufs=1))
        ptp = ctx.enter_context(tc.tile_pool(name="ptp", bufs=pt_bufs))
        fin = ctx.enter_context(tc.tile_pool(name="fin", bufs=2))
        ps_att = ctx.enter_context(tc.tile_pool(name="psatt", bufs=att_bufs,
                                                space="PSUM"))
        ps_oa = ctx.enter_context(tc.tile_pool(name="psoa", bufs=1,
                                               space="PSUM"))
        ps_ab = ctx.enter_context(tc.tile_pool(name="psab", bufs=ab_bufs,
                                               space="PSUM"))

        with tc.high_priority():
            ident = singles.tile([128, 128], BF16)
            make_identity(nc, ident[:, :])
            # PE warmup during the first x transfer
            warm = ps_att.tile([128, BPC, 512], F32, tag="att")
            for i in range(6):
                nc.tensor.matmul(warm[:, i % 2, 0:128], ident[:, :],
                                 ident[:, :], start=True, stop=True)

        # fp32 weight staging via HWDGE, pack on Pool. b0 packs [Wq|Wk],
        # b1 [Wk|Wq] so q lands on the batch's own row half with a plain
        # copy (same as v1).
        wq_s = singles.tile([128, CB, 64], F32)
        wk_s = singles.tile([128, CB, 64], F32)
        wv_s = singles.tile([128, CB, 64], F32)
        nc.sync.dma_start(out=wq_s[:, :, :],
                          in_=wq_d.rearrange("(c p) h -> p c h", p=128))
        nc.sync.dma_start(out=wk_s[:, :, :],
                          in_=wk_d.rearrange("(c p) h -> p c h", p=128))
        nc.sync.dma_start(out=wv_s[:, :, :],
                          in_=wv_d.rearrange("(c p) h -> p c h", p=128))
        wqk0 = singles.tile([128, CB, 128], BF16)
        nc.gpsimd.tensor_copy(wqk0[:, :, 0:64], wq_s[:, :, :])
        nc.gpsimd.tensor_copy(wqk0[:, :, 64:128], wk_s[:, :, :])
        wqk1 = singles.tile([128, CB, 128], BF16)
        nc.gpsimd.tensor_copy(wqk1[:, :, 0:64], wk_s[:, :, :])
        nc.gpsimd.tensor_copy(wqk1[:, :, 64:128], wq_s[:, :, :])
        wqk = (wqk0, wqk1)
        wv_b = singles.tile([128, CB, 64], BF16)
        nc.gpsimd.tensor_copy(wv_b[:, :, :], wv_s[:, :, :])

        def body():
            # q^T for both batches: qk0 rows 0:64 = q0, qk1 rows 64:128 = q1
            qk0 = qkp.tile([128, T], BF16, tag="qk0")
            qk1 = qkp.tile([128, T], BF16, tag="qk1")
            # k^T for both batches: rows 0:64 = k0, rows 64:128 = k1
            skk = qkp.tile([128, T], BF16, tag="skk")
            vn0 = vnp.tile([128, TT, 65], BF16, tag="vn0")
            vn1 = vnp.tile([128, TT, 65], BF16, tag="vn1")
            nc.vector.memset(vn0[:, :, 64], 1.0)
            nc.vector.memset(vn1[:, :, 64], 1.0)
            qkt = (qk0, qk1)
            vnt = (vn0, vn1)

            def phase_ab(blk):
                xTs = []
                for b in range(BPC):
                    xT = xTp.tile([128, CB, 512], BF16, tag=f"xT{b}")
                    xTs.append(xT)
                    xt = xp.tile([128, 4, C], BF16, tag="x")
                    base = blk * 512
                    if blk == 0 and b == 0:
                        with tc.high_priority(offset=1 << 20):
                            nc.gpsimd.dma_start(
                                out=xt[:, 0:2, :],
                                in_=x_d[b, base:base + 256, :].rearrange(
                                    "(f p) c -> p f c", p=128))
                        with tc.high_priority():
                            nc.gpsimd.dma_start(
                                out=xt[:, 2:4, :],
                                in_=x_d[b, base + 256:base + 512, :].rearrange(
                                    "(f p) c -> p f c", p=128))
                    else:
                        nc.gpsimd.dma_start(
                            out=xt[:, :, :],
                            in_=x_d[b, base:base + 512, :].rearrange(
                                "(f p) c -> p f c", p=128))
                    for t4 in range(4):
                        ptr = ps_ab.tile([128, CB, 128], BF16, tag="ab")
                        for ci in range(CB):
                            nc.tensor.matmul(ptr[:, ci, :],
                                             xt[:, t4, ci * 128:(ci + 1) * 128],
                                             ident[:, :], is_transpose=True)
                        nc.vector.tensor_copy(
                            xT[:, :, t4 * 128:(t4 + 1) * 128], ptr[:, :, :])
                sl = slice(blk * 512, (blk + 1) * 512)
                # Projections with split-K: each 128-row chunk becomes lo/hi
                # 64-row matmuls at tile positions (0,0)/(64,0) accumulating
                # into two separate PSUM banks (measured 2.2x the K=128
                # stream rate); a DVE add fuses the partials straight to
                # bf16. v partials borrow the oa banks (idle during AB) so
                # the ab ring never waits on the adds.
                for b in range(BPC):
                    if qsplit:
                        pqA = ps_ab.tile([128, 512], F32, tag="ab")
                        pqB = ps_ab.tile([128, 512], F32, tag="ab")
                        for ci in range(CB):
                            nc.tensor.matmul(pqA[:, :], wqk[b][0:64, ci, :],
                                             xTs[b][0:64, ci, :],
                                             start=(ci == 0),
                                             stop=(ci == CB - 1),
                                             tile_position=(0, 0))
                            nc.tensor.matmul(pqB[:, :], wqk[b][64:128, ci, :],
                                             xTs[b][64:128, ci, :],
                                             start=(ci == 0),
                                             stop=(ci == CB - 1),
                                             tile_position=(64, 0))
                        # walrus rejects dual-PSUM-input DVE ops: stage A
                        # to the SBUF destination, then add B in place
                        nc.vector.tensor_copy(qkt[b][:, sl], pqA[:, :])
                        nc.vector.scalar_tensor_tensor(
                            out=qkt[b][:, sl], in0=qkt[b][:, sl], scalar=0.0,
                            in1=pqB[:, :], op0=mybir.AluOpType.add,
                            op1=mybir.AluOpType.add)
                    else:
                        pq = ps_ab.tile([128, 512], F32, tag="ab")
                        for ci in range(CB):
                            nc.tensor.matmul(pq[:, :], wqk[b][:, ci, :],
                                             xTs[b][:, ci, :],
                                             start=(ci == 0),
                                             stop=(ci == CB - 1))
                        nc.vector.tensor_copy(qkt[b][:, sl], pq[:, :])
                    if vsplit:
                        # v in natural [t, h]: same split, partials in the
                        # oa banks (idle during AB)
                        pvvA = ps_oa.tile([128, 4, 64], F32, tag="oa0")
                        pvvB = ps_oa.tile([128, 4, 64], F32, tag="oa1")
                        for t4 in range(4):
                            ts = slice(t4 * 128, (t4 + 1) * 128)
                            for ci in range(CB):
                                nc.tensor.matmul(pvvA[:, t4, :],
                                                 xTs[b][0:64, ci, ts],
                                                 wv_b[0:64, ci, :],
                                                 start=(ci == 0),
                                                 stop=(ci == CB - 1),
                                                 tile_position=(0, 0),
                                                 skip_group_check=True)
                                nc.tensor.matmul(pvvB[:, t4, :],
                                                 xTs[b][64:128, ci, ts],
                                                 wv_b[64:128, ci, :],
                                                 start=(ci == 0),
                                                 stop=(ci == CB - 1),
                                                 tile_position=(64, 0),
                                                 skip_group_check=True)
                        vsl = vnt[b][:, blk * 4:(blk + 1) * 4, 0:64]
                        nc.vector.tensor_copy(vsl, pvvA[:, :, :])
                        nc.vector.scalar_tensor_tensor(
                            out=vsl, in0=vsl, scalar=0.0,
                            in1=pvvB[:, :, :], op0=mybir.AluOpType.add,
                            op1=mybir.AluOpType.add)
                    else:
                        pvv = ps_ab.tile([128, 4, 64], F32, tag="ab")
                        for t4 in range(4):
                            ts = slice(t4 * 128, (t4 + 1) * 128)
                            for ci in range(CB):
                                nc.tensor.matmul(pvv[:, t4, :],
                                                 xTs[b][:, ci, ts],
                                                 wv_b[:, ci, :],
                                                 start=(ci == 0),
                                                 stop=(ci == CB - 1))
                        nc.vector.tensor_copy(
                            vnt[b][:, blk * 4:(blk + 1) * 4, 0:64],
                            pvv[:, :, :])
                # k partition shifts into the shared k tile
                nc.sync.dma_start(out=skk[0:64, sl], in_=qk0[64:128, sl])
                nc.sync.dma_start(out=skk[64:128, sl], in_=qk1[0:64, sl])

            def phase_c(bi):
                # one full 2KB bank per batch: a single bank-wide PSUM
                # accumulation group (start on the very first matmul into the
                # bank, stop on the last). Regions zero lazily on first
                # touch; interleaved per-region start/stops would corrupt
                # neighbours (pending-zero arms the whole 2KB zero region).
                oacc0 = ps_oa.tile([128, 4, 128], F32, tag="oa0")
                oacc1 = ps_oa.tile([128, 4, 128], F32, tag="oa1")
                oaccs = (oacc0, oacc1)
                last = 4 * bi + 3

                def geom(j):
                    r = j - 4 * bi
                    w, c0 = (512, 0) if r <= 0 else (512 - 128 * r, 128 * r)
                    return r, w, c0

                def emit_qk(j):
                    r, w, c0 = geom(j)
                    js = slice(j * 128, (j + 1) * 128)
                    cs = slice(bi * 512 + c0, (bi + 1) * 512)
                    sab = ps_att.tile([128, BPC, 512], F32, tag="att")
                    nc.tensor.matmul(sab[:, 0, 0:w], skk[0:64, js],
                                     qk0[0:64, cs], start=True, stop=True,
                                     tile_position=(0, 0))
                    nc.tensor.matmul(sab[:, 1, 0:w], skk[64:128, js],
                                     qk1[64:128, cs], start=True, stop=True,
                                     tile_position=(64, 0))
                    return sab

                sab = emit_qk(0)
                for j in range(last + 1):
                    r, w, c0 = geom(j)
                    pt = ptp.tile([128, BPC, 512], BF16, tag="pt")
                    nc.scalar.activation(pt[:, :, 0:w], sab[:, :, 0:w],
                                         mybir.ActivationFunctionType.Exp,
                                         scale=SCALE)
                    if r >= 0:
                        nc.gpsimd.affine_select(
                            out=pt[:, :, 0:128], in_=pt[:, :, 0:128],
                            compare_op=mybir.AluOpType.is_ge, fill=0.0,
                            base=0, pattern=[[0, BPC], [1, 128]],
                            channel_multiplier=-1)
                    if j < last:
                        sab = emit_qk(j + 1)
                    # PV: stationary = scores tile (s x t), moving = v|ones
                    for tau in range(max(r, 0), 4):
                        p0 = tau * 128 - c0
                        for b in range(BPC):
                            nc.tensor.matmul(oaccs[b][:, tau, 0:65],
                                             pt[:, b, p0:p0 + 128],
                                             vnt[b][:, j, :],
                                             start=(j == 0 and tau == max(r, 0)),
                                             stop=(j == last and tau == 3),
                                             skip_group_check=True)
                for b in range(BPC):
                    ot = fin.tile([128, 4, 65], F32, tag="ot")
                    if bi == NB - 1 and b == 1:
                        nc.scalar.copy(ot[:, :, :], oaccs[b][:, :, 0:65])
                    else:
                        nc.vector.tensor_copy(ot[:, :, :], oaccs[b][:, :, 0:65])
                    yt = fin.tile([128, 4, 64], F32, tag="yt")
                    if bi < NB - 1:
                        for t4 in range(4):
                            nc.gpsimd.normalize_recip(yt[:, t4, :],
                                                      ot[:, t4, 0:64],
                                                      ot[:, t4, 64:65])
                    else:
                        linv = fin.tile([128, 4], F32, tag="linv")
                        nc.vector.reciprocal(linv[:, :], ot[:, :, 64])
                        for t4 in range(4):
                            nc.vector.tensor_scalar_mul(yt[:, t4, :],
                                                        ot[:, t4, 0:64],
                                                        linv[:, t4:t4 + 1])
                    nc.sync.dma_start(
                        out=y_d[b, bi * 512:(bi + 1) * 512, :].rearrange(
                            "(f p) h -> p f h", p=128),
                        in_=yt[:, :, :])

            for blk in range(NB):
                phase_ab(blk)
                phase_c(blk)

        if reps == 1:
            body()
        else:
            with tc.For_i(0, reps, 1):
                body()

    nc.compile()
    return nc


_CACHE = {}


def _get_program(**kw):
    key = tuple(sorted(kw.items()))
    if key not in _CACHE:
        kw2 = dict(kw)
        v = kw2.pop("v", 2)
        fn = build_program_v2 if int(v) == 2 else build_program
        _CACHE[key] = fn(**kw2)
    return _CACHE[key]


def run_sharded(x, Wq, Wk, Wv, trace=False, **build_kw):
    """Run on 8 cores, return (y_full, BassKernelResults)."""
    nc = _get_program(**build_kw)
    x = np.ascontiguousarray(np.asarray(x, dtype=np.float32))
    Wq = np.ascontiguousarray(np.asarray(Wq, dtype=np.float32))
    Wk = np.ascontiguousarray(np.asarray(Wk, dtype=np.float32))
    Wv = np.ascontiguousarray(np.asarray(Wv, dtype=np.float32))
    xs = x.reshape(NCORES, BPC, T, C)
    in_maps = [{"x": np.ascontiguousarray(xs[i]), "Wq": Wq, "Wk": Wk, "Wv": Wv}
               for i in range(NCORES)]
    res = run_bass_kernel_spmd(nc, in_maps, list(range(NCORES)), trace=trace)
    y = np.stack([res.results[i]["y"] for i in range(NCORES)], axis=0)
    return y.reshape(B, T, H), res


def kernel(x, Wq, Wk, Wv):
    y, _ = run_sharded(x, Wq, Wk, Wv, trace=False)
    return y


# ---------------- timing support (no NTFF profiler in this container) ----


def make_runner(nc, n_iter=1):
    """Build a reusable sharded jit callable for `nc` (mirrors
    bass2jax.run_bass_via_pjrt's multi-core path, without donation so
    device inputs can be reused across timed calls)."""
    import jax
    from jax.sharding import Mesh, PartitionSpec
    try:
        from jax.experimental.shard_map import shard_map
    except ImportError:  # newer jax
        from jax.shard_map import shard_map
    from concourse import bass2jax
    bass2jax.install_neuronx_cc_hook()

    part_name = (nc.partition_id_tensor.name if nc.partition_id_tensor
                 else None)
    in_names, out_names, out_avals, zero_outs = [], [], [], []
    for alloc in nc.m.functions[0].allocations:
        if not isinstance(alloc, mybir.MemoryLocationSet):
            continue
        name = alloc.memorylocations[0].name
        if alloc.kind == "ExternalInput":
            if name != part_name:
                in_names.append(name)
        elif alloc.kind == "ExternalOutput":
            out_names.append(name)
            shape = tuple(alloc.tensor_shape)
            dtype = mybir.dt.np(alloc.dtype)
            out_avals.append(jax.core.ShapedArray(shape, dtype))
            zero_outs.append(np.zeros(shape, dtype))
    n_params = len(in_names)
    all_names = in_names + out_names
    if part_name is not None:
        all_names = all_names + [part_name]

    def _body(*args):
        ins = list(args[:n_params])
        youts = list(args[n_params:n_params + len(out_names)])
        for _ in range(n_iter):
            operands = ins + youts
            if part_name is not None:
                operands.append(bass2jax.partition_id_tensor())
            outs = bass2jax._bass_exec_p.bind(
                *operands, out_avals=tuple(out_avals),
                in_names=tuple(all_names), out_names=tuple(out_names),
                lowering_input_output_aliases=(),
                sim_require_finite=True, sim_require_nnan=True, nc=nc)
            youts = list(outs)
        return tuple(youts)

    devices = jax.devices()[:NCORES]
    mesh = Mesh(np.asarray(devices), ("core",))
    in_specs = (PartitionSpec("core"),) * (n_params + len(out_names))
    out_specs = (PartitionSpec("core"),) * len(out_names)
    fn = jax.jit(shard_map(_body, mesh=mesh, in_specs=in_specs,
                           out_specs=out_specs, check_rep=False),
                 keep_unused=True)
    return fn, in_names, zero_outs, mesh


def _timed_calls(fn, dev_in, iters):
    import time as _time
    import jax
    out = fn(*dev_in)
    jax.block_until_ready(out)
    ts = []
    for _ in range(iters):
        t0 = _time.perf_counter_ns()
        out = fn(*dev_in)
        jax.block_until_ready(out)
        ts.append(_time.perf_counter_ns() - t0)
    ts.sort()
    return ts


def time_calls(nc, in_maps, iters=10):
    """Sorted wall times (ns) of warm sharded calls of nc's NEFF."""
    import jax
    from jax.sharding import NamedSharding, PartitionSpec
    fn, in_names, zero_outs, mesh = make_runner(nc, n_iter=1)
    sh = NamedSharding(mesh, PartitionSpec("core"))
    concat = [np.concatenate([np.asarray(m[n]) for m in in_maps], axis=0)
              for n in in_names]
    concat += [np.zeros((NCORES * z.shape[0], *z.shape[1:]), z.dtype)
               for z in zero_outs]
    dev_in = [jax.device_put(a, sh) for a in concat]
    return _timed_calls(fn, dev_in, iters)


_BASELINE = {}


def baseline_nc():
    """Tiny kernel to measure the axon dispatch floor."""
    if "nc" in _BASELINE:
        return _BASELINE["nc"]
    nc = bacc.Bacc("TRN2", target_bir_lowering=False, debug=False,
                   num_devices=NCORES)
    a = nc.dram_tensor("a", [128, 128], F32, kind="ExternalInput").ap()
    b = nc.dram_tensor("b", [128, 128], F32, kind="ExternalOutput").ap()
    with tile.TileContext(nc) as tc:
        with tc.tile_pool(name="p", bufs=1) as pool:
            t = pool.tile([128, 128], F32)
            nc.sync.dma_start(out=t[:, :], in_=a)
            nc.sync.dma_start(out=b, in_=t[:, :])
    nc.compile()
    _BASELINE["nc"] = nc
    return nc



# revision 13
# speedup vs baseline: 1.0830x; 1.0259x over previous
"""Single-head causal attention on 8 trn2 NeuronCores.

Problem: x[16, 2048, 1024] fp32, Wq/Wk/Wv[1024, 64] fp32 ->
         out[16, 2048, 64] = softmax(causal(q k^T / sqrt(64))) v

Sharding: data-parallel over batch B=16 -> 2 batches per core, no
collectives. SPMD program; the two per-core batches are processed in
lockstep so small-dim matmuls can be packed across them.

v2 structure (build_program_v2, the default; ~96us/rep sustained vs
~114us for v1 on the same protocol, REPS=257 slope):
  * x loaded with gpsimd casting DMAs (fp32 HBM -> bf16 SBUF); x^T via
    PE transposes (1 cyc/row bf16), wide DVE copy-outs.
  * Projections: per-batch packed [Wq|Wk]/[Wk|Wq] -> q^T/k^T; k halves
    partition-shifted into a shared skk tile by SBUF->SBUF DMA.
  * v computed directly in NATURAL [T, 64] layout: stationary = xT
    chunk, moving = Wv (64-col streams run at full rate; weight reloads
    hide under streams). No v^T->v transposes.
  * Scores S^T: two batches' K=64 matmuls concurrent via row tiling
    (0,0)/(64,0) -- genuine 2x since each only half-fills the array.
  * PV swapped: stationary = exp-scores tile [s,t-tile], moving =
    v|ones [s,65] -> output lands NATURAL [t, 64+denominator]: no
    finalize transposes; normalize is a per-partition scalar divide.
  * PV accumulation: ONE bank-wide PSUM group per (batch, block)
    (start on first matmul into the bank, stop on last). HW pending-
    zero arms whole 2KB zero-regions; interleaved per-region groups in
    a shared bank corrupt each other (measured + interp-confirmed).
  * Causal mask: affine_select on Pool over the first 128 columns of
    diagonal tiles only.

Floor analysis: PE MAC floor = QK 32.8k + V 16.4k + scores 17.4k
(row-tile paired) + PV 17.7k + transposes 27.6k ~= 112k cycles; the
sustained PE clock is ~1.2GHz (short bursts boost to ~2.4-2.8GHz,
which is why REPS=33 slopes are unstable; use REPS=257).
Measured dead ends: fp8e4 DoubleRow runs SLOWER than bf16 on HW
despite the cost model 0.5 cyc/row; split-K pairing (K=128 -> 2x64 in
two banks) is a wash (pair concurrency just recovers the full-array
rate, total columns unchanged); 4-way 32-row tiling worse than 2-way;
dual-PSUM-input scalar_tensor_tensor crashes walrus; same-region
accumulation across tile positions faults the device.
"""

import sys

sys.path.insert(0, "/opt/trn_rl_repo")

import numpy as np

import concourse.bass as bass  # noqa: F401
import concourse.bacc as bacc
import concourse.mybir as mybir
import concourse.tile as tile
from concourse.masks import make_identity
from concourse.bass_utils import run_bass_kernel_spmd

B, T, C, H = 16, 2048, 1024, 64
NCORES = 8
BPC = B // NCORES  # batches per core
CB = C // 128      # 8 contraction chunks
TT = T // 128      # 16 T tiles of 128
NB = T // 512      # 4 T blocks of 512
F32 = mybir.dt.float32
BF16 = mybir.dt.bfloat16
SCALE = float(H) ** -0.5


def build_program(reps=1, att_bufs=2, ab_bufs=2, pt_bufs=3, x_bufs=3,
                  xt_bufs=2, oex_bufs=2):
    from contextlib import ExitStack

    nc = bacc.Bacc("TRN2", target_bir_lowering=False, debug=False,
                   num_devices=NCORES)
    x_d = nc.dram_tensor("x", [BPC, T, C], F32, kind="ExternalInput").ap()
    wq_d = nc.dram_tensor("Wq", [C, H], F32, kind="ExternalInput").ap()
    wk_d = nc.dram_tensor("Wk", [C, H], F32, kind="ExternalInput").ap()
    wv_d = nc.dram_tensor("Wv", [C, H], F32, kind="ExternalInput").ap()
    y_d = nc.dram_tensor("y", [BPC, T, H], F32, kind="ExternalOutput").ap()

    with tile.TileContext(nc) as tc, ExitStack() as ctx:
        singles = ctx.enter_context(tc.tile_pool(name="singles", bufs=1))
        xp = ctx.enter_context(tc.tile_pool(name="xp", bufs=x_bufs))
        xTp = ctx.enter_context(tc.tile_pool(name="xTp", bufs=xt_bufs))
        qkp = ctx.enter_context(tc.tile_pool(name="qkp", bufs=1))
        vnp = ctx.enter_context(tc.tile_pool(name="vnp", bufs=1))
        vTp = ctx.enter_context(tc.tile_pool(name="vTp", bufs=2))
        ptp = ctx.enter_context(tc.tile_pool(name="ptp", bufs=pt_bufs))
        oxp = ctx.enter_context(tc.tile_pool(name="oxp", bufs=oex_bufs))
        fin = ctx.enter_context(tc.tile_pool(name="fin", bufs=2))
        ps_att = ctx.enter_context(tc.tile_pool(name="psatt", bufs=att_bufs,
                                                space="PSUM"))
        ps_oa = ctx.enter_context(tc.tile_pool(name="psoa", bufs=1,
                                               space="PSUM"))
        ps_ab = ctx.enter_context(tc.tile_pool(name="psab", bufs=ab_bufs,
                                               space="PSUM"))

        # identity build is dependency-free Pool work; run it before the x
        # descriptor generation so the first transposes aren't blocked on it
        with tc.high_priority():
            ident = singles.tile([128, 128], BF16)
            make_identity(nc, ident[:, :])
            identf = singles.tile([128, 128], F32)
            make_identity(nc, identf[:, :])
            # warm the PE array (HAM clock ramp) with dummy matmuls while
            # the first x transfer is in flight; they borrow an attention
            # ring slot that isn't needed until ~13us in
            warm = ps_att.tile([128, BPC, 512], F32, tag="att")
            for i in range(24):
                nc.tensor.matmul(warm[:, i % 2, 0:128], ident[:, :],
                                 ident[:, :], start=True, stop=True)

        # Weights: fp32 staging via HWDGE (keeps Pool free for the x casting
        # DMAs at startup), then convert/pack on gpsimd. b0 packs [Wq|Wk],
        # b1 [Wk|Wq] so q lands on the batch's own row half with a plain
        # copy.
        wq_s = singles.tile([128, CB, 64], F32)
        wk_s = singles.tile([128, CB, 64], F32)
        wv_s = singles.tile([128, CB, 64], F32)
        nc.sync.dma_start(out=wq_s[:, :, :],
                          in_=wq_d.rearrange("(c p) h -> p c h", p=128))
        nc.sync.dma_start(out=wk_s[:, :, :],
                          in_=wk_d.rearrange("(c p) h -> p c h", p=128))
        nc.sync.dma_start(out=wv_s[:, :, :],
                          in_=wv_d.rearrange("(c p) h -> p c h", p=128))
        # packing on Pool: these wait on the weight transfers, and on DVE
        # they would head-of-line block the first transpose copies
        wqk0 = singles.tile([128, CB, 128], BF16)
        nc.gpsimd.tensor_copy(wqk0[:, :, 0:64], wq_s[:, :, :])
        nc.gpsimd.tensor_copy(wqk0[:, :, 64:128], wk_s[:, :, :])
        wqk1 = singles.tile([128, CB, 128], BF16)
        nc.gpsimd.tensor_copy(wqk1[:, :, 0:64], wk_s[:, :, :])
        nc.gpsimd.tensor_copy(wqk1[:, :, 64:128], wq_s[:, :, :])
        wv2 = singles.tile([128, CB, 128], BF16)
        nc.gpsimd.tensor_copy(wv2[:, :, 0:64], wv_s[:, :, :])
        nc.gpsimd.tensor_copy(wv2[:, :, 64:128], wv_s[:, :, :])
        wqk = (wqk0, wqk1)

        def body():
            # q^T for both batches: qk0 rows 0:64 = q0, qk1 rows 64:128 = q1
            qk0 = qkp.tile([128, T], BF16, tag="qk0")
            qk1 = qkp.tile([128, T], BF16, tag="qk1")
            # k^T for both batches: rows 0:64 = k0, rows 64:128 = k1
            skk = qkp.tile([128, T], BF16, tag="skk")
            vn0 = vnp.tile([128, TT, 65], BF16, tag="vn0")
            vn1 = vnp.tile([128, TT, 65], BF16, tag="vn1")
            nc.vector.memset(vn0[:, :, 64], 1.0)
            nc.vector.memset(vn1[:, :, 64], 1.0)
            qkt = (qk0, qk1)
            vnt = (vn0, vn1)

            def phase_ab(blk):
                """Load + transpose x, projections for T block blk."""
                xTs = []
                for b in range(BPC):
                    xT = xTp.tile([128, CB, 512], BF16, tag=f"xT{b}")
                    xTs.append(xT)
                    # one casting DMA per (batch, block): 1 MB fp32 -> bf16.
                    # The very first block is split in two so the transpose
                    # pipeline fills sooner.
                    xt = xp.tile([128, 4, C], BF16, tag="x")
                    base = blk * 512
                    if blk == 0 and b == 0:
                        # first load split in two and boosted ahead of
                        # everything (even the identity build: it hides
                        # under the transfer) so the pipeline fills asap
                        with tc.high_priority(offset=1 << 20):
                            nc.gpsimd.dma_start(
                                out=xt[:, 0:2, :],
                                in_=x_d[b, base:base + 256, :].rearrange(
                                    "(f p) c -> p f c", p=128))
                        with tc.high_priority():
                            nc.gpsimd.dma_start(
                                out=xt[:, 2:4, :],
                                in_=x_d[b, base + 256:base + 512, :].rearrange(
                                    "(f p) c -> p f c", p=128))
                    else:
                        nc.gpsimd.dma_start(
                            out=xt[:, :, :],
                            in_=x_d[b, base:base + 512, :].rearrange(
                                "(f p) c -> p f c", p=128))
                    for t4 in range(4):
                        ptr = ps_ab.tile([128, CB, 128], BF16, tag="ab")
                        for ci in range(CB):
                            nc.tensor.matmul(ptr[:, ci, :],
                                             xt[:, t4, ci * 128:(ci + 1) * 128],
                                             ident[:, :], is_transpose=True)
                        nc.vector.tensor_copy(
                            xT[:, :, t4 * 128:(t4 + 1) * 128], ptr[:, :, :])
                sl = slice(blk * 512, (blk + 1) * 512)
                for b in range(BPC):
                    pq = ps_ab.tile([128, 512], F32, tag="ab")
                    for ci in range(CB):
                        nc.tensor.matmul(pq[:, :], wqk[b][:, ci, :],
                                         xTs[b][:, ci, :],
                                         start=(ci == 0), stop=(ci == CB - 1))
                    nc.vector.tensor_copy(qkt[b][:, sl], pq[:, :])
                # k partition shifts into the shared k tile
                nc.sync.dma_start(out=skk[0:64, sl], in_=qk0[64:128, sl])
                nc.sync.dma_start(out=skk[64:128, sl], in_=qk1[0:64, sl])
                # v for both batches, column-tiled into one PSUM bank
                pv = ps_ab.tile([128, 512], F32, tag="ab")
                for ci in range(CB):
                    nc.tensor.matmul(pv[0:64, :], wv2[:, ci, 0:64],
                                     xTs[0][:, ci, :], start=(ci == 0),
                                     stop=(ci == CB - 1), tile_position=(0, 0))
                    nc.tensor.matmul(pv[64:128, :], wv2[:, ci, 64:128],
                                     xTs[1][:, ci, :], start=(ci == 0),
                                     stop=(ci == CB - 1), tile_position=(0, 64))
                vT = vTp.tile([128, 512], BF16, tag="vT")
                nc.vector.tensor_copy(vT[:, :], pv[:, :])
                for b in range(BPC):
                    pvn = ps_ab.tile([128, 4, 64], BF16, tag="ab")
                    bs = slice(b * 64, (b + 1) * 64)
                    for t4 in range(4):
                        nc.tensor.matmul(
                            pvn[:, t4, :],
                            vT[bs, t4 * 128:(t4 + 1) * 128],
                            ident[bs, bs], is_transpose=True)
                    nc.vector.tensor_copy(
                        vnt[b][:, blk * 4:(blk + 1) * 4, 0:64], pvn[:, :, :])

            def phase_c(bi):
                """Attention for T-row block bi (needs k/v blocks <= bi)."""
                oacc = ps_oa.tile([65, BPC, 512], F32, tag="oa")
                last = 4 * bi + 3

                def geom(j):
                    r = j - 4 * bi
                    w, c0 = (512, 0) if r <= 0 else (512 - 128 * r, 128 * r)
                    return r, w, c0

                def emit_qk(j):
                    r, w, c0 = geom(j)
                    js = slice(j * 128, (j + 1) * 128)
                    cs = slice(bi * 512 + c0, (bi + 1) * 512)
                    sab = ps_att.tile([128, BPC, 512], F32, tag="att")
                    nc.tensor.matmul(sab[:, 0, 0:w], skk[0:64, js],
                                     qk0[0:64, cs], start=True, stop=True,
                                     tile_position=(0, 0))
                    nc.tensor.matmul(sab[:, 1, 0:w], skk[64:128, js],
                                     qk1[64:128, cs], start=True, stop=True,
                                     tile_position=(64, 0))
                    return sab

                # software-pipelined: QK(j+1) is emitted before PV(j) so the
                # PE queue never has a PV (waiting on exp) ahead of ready QK
                sab = emit_qk(0)
                for j in range(last + 1):
                    r, w, c0 = geom(j)
                    pt = ptp.tile([128, BPC, 512], BF16, tag="pt")
                    nc.scalar.activation(pt[:, :, 0:w], sab[:, :, 0:w],
                                         mybir.ActivationFunctionType.Exp,
                                         scale=SCALE)
                    if r >= 0:
                        # keep where within-tile free idx >= partition idx.
                        # Since partitions only span 0..127, columns >= 128
                        # always pass: mask just the first 128 columns.
                        nc.gpsimd.affine_select(
                            out=pt[:, :, 0:128], in_=pt[:, :, 0:128],
                            compare_op=mybir.AluOpType.is_ge, fill=0.0,
                            base=0, pattern=[[0, BPC], [1, 128]],
                            channel_multiplier=-1)
                    if j < last:
                        sab = emit_qk(j + 1)
                    for b in range(BPC):
                        nc.tensor.matmul(oacc[:, b, c0:512], vnt[b][:, j, :],
                                         pt[:, b, 0:w], start=(j == 0),
                                         stop=(j == last))
                for b in range(BPC):
                    oex = oxp.tile([65, 512], F32, tag="oex")
                    # on the last block ACT is idle after its final exp; give
                    # it batch 1's drain so the two finalize chains overlap
                    if bi == NB - 1 and b == 1:
                        nc.scalar.copy(oex[:, :], oacc[:, b, :])
                    else:
                        nc.vector.tensor_copy(oex[:, :], oacc[:, b, :])
                    # NOTE: must NOT share the "ab" ring — that would make
                    # block k+1's transposes wait on this finalize
                    pso = ps_att.tile([128, 4, 65], F32, tag="att")
                    for t4 in range(4):
                        nc.tensor.matmul(pso[:, t4, :],
                                         oex[0:65, t4 * 128:(t4 + 1) * 128],
                                         identf[0:65, 0:65],
                                         is_transpose=True)
                    ot = fin.tile([128, 4, 65], F32, tag="ot")
                    if bi == NB - 1 and b == 1:
                        nc.scalar.copy(ot[:, :, :], pso[:, :, :])
                    else:
                        nc.vector.tensor_copy(ot[:, :, :], pso[:, :, :])
                    yt = fin.tile([128, 4, 64], F32, tag="yt")
                    if bi < NB - 1:
                        for t4 in range(4):
                            # out = ot / l on gpsimd; overwrites the l
                            # column with its reciprocal (unused afterwards)
                            nc.gpsimd.normalize_recip(yt[:, t4, :],
                                                      ot[:, t4, 0:64],
                                                      ot[:, t4, 64:65])
                    else:
                        # last block: Pool's serial ISA ops would sit on the
                        # kernel tail; DVE is idle there
                        linv = fin.tile([128, 4], F32, tag="linv")
                        nc.vector.reciprocal(linv[:, :], ot[:, :, 64])
                        for t4 in range(4):
                            nc.vector.tensor_scalar_mul(yt[:, t4, :],
                                                        ot[:, t4, 0:64],
                                                        linv[:, t4:t4 + 1])
                    nc.sync.dma_start(
                        out=y_d[b, bi * 512:(bi + 1) * 512, :].rearrange(
                            "(f p) h -> p f h", p=128),
                        in_=yt[:, :, :])

            for blk in range(NB):
                phase_ab(blk)
                phase_c(blk)

        if reps == 1:
            body()
        else:
            with tc.For_i(0, reps, 1):
                body()

    nc.compile()
    return nc


def build_program_v2(reps=1, att_bufs=2, ab_bufs=2, pt_bufs=3, x_bufs=3,
                     xt_bufs=2, qsplit=False, vsplit=False):
    """v2: natural-layout V (stationary=xT tile, moving=Wv) and swapped PV
    (stationary=scores tile, moving=v|ones) so attention output lands in
    natural [T, H] layout with the softmax denominator as a free 65th
    column -> no vn/finalize transposes. Scores stay bf16 row-tiled pairs
    (fp8 DoubleRow measured SLOWER than bf16 on HW despite the cost model's
    0.5 cyc/row: 333.8ns vs 213.4ns per row-tiled pair)."""
    from contextlib import ExitStack

    nc = bacc.Bacc("TRN2", target_bir_lowering=False, debug=False,
                   num_devices=NCORES)
    x_d = nc.dram_tensor("x", [BPC, T, C], F32, kind="ExternalInput").ap()
    wq_d = nc.dram_tensor("Wq", [C, H], F32, kind="ExternalInput").ap()
    wk_d = nc.dram_tensor("Wk", [C, H], F32, kind="ExternalInput").ap()
    wv_d = nc.dram_tensor("Wv", [C, H], F32, kind="ExternalInput").ap()
    y_d = nc.dram_tensor("y", [BPC, T, H], F32, kind="ExternalOutput").ap()

    with tile.TileContext(nc) as tc, ExitStack() as ctx:
        singles = ctx.enter_context(tc.tile_pool(name="singles", bufs=1))
        xp = ctx.enter_context(tc.tile_pool(name="xp", bufs=x_bufs))
        xTp = ctx.enter_context(tc.tile_pool(name="xTp", bufs=xt_bufs))
        qkp = ctx.enter_context(tc.tile_pool(name="qkp", bufs=1))
        vnp = ctx.enter_context(tc.tile_pool(name="vnp", bufs=1))
        ptp = ctx.enter_context(tc.tile_pool(name="ptp", bufs=pt_bufs))
        fin = ctx.enter_context(tc.tile_pool(name="fin", bufs=2))
        ps_att = ctx.enter_context(tc.tile_pool(name="psatt", bufs=att_bufs,
                                                space="PSUM"))
        ps_oa = ctx.enter_context(tc.tile_pool(name="psoa", bufs=1,
                                               space="PSUM"))
        ps_ab = ctx.enter_context(tc.tile_pool(name="psab", bufs=ab_bufs,
                                               space="PSUM"))

        with tc.high_priority():
            ident = singles.tile([128, 128], BF16)
            make_identity(nc, ident[:, :])
            # PE warmup during the first x transfer
            warm = ps_att.tile([128, BPC, 512], F32, tag="att")
            for i in range(24):
                nc.tensor.matmul(warm[:, i % 2, 0:128], ident[:, :],
                                 ident[:, :], start=True, stop=True)

        # fp32 weight staging via HWDGE, pack on Pool. b0 packs [Wq|Wk],
        # b1 [Wk|Wq] so q lands on the batch's own row half with a plain
        # copy (same as v1).
        wq_s = singles.tile([128, CB, 64], F32)
        wk_s = singles.tile([128, CB, 64], F32)
        wv_s = singles.tile([128, CB, 64], F32)
        nc.sync.dma_start(out=wq_s[:, :, :],
                          in_=wq_d.rearrange("(c p) h -> p c h", p=128))
        nc.sync.dma_start(out=wk_s[:, :, :],
                          in_=wk_d.rearrange("(c p) h -> p c h", p=128))
        nc.sync.dma_start(out=wv_s[:, :, :],
                          in_=wv_d.rearrange("(c p) h -> p c h", p=128))
        wqk0 = singles.tile([128, CB, 128], BF16)
        nc.gpsimd.tensor_copy(wqk0[:, :, 0:64], wq_s[:, :, :])
        nc.gpsimd.tensor_copy(wqk0[:, :, 64:128], wk_s[:, :, :])
        wqk1 = singles.tile([128, CB, 128], BF16)
        nc.gpsimd.tensor_copy(wqk1[:, :, 0:64], wk_s[:, :, :])
        nc.gpsimd.tensor_copy(wqk1[:, :, 64:128], wq_s[:, :, :])
        wqk = (wqk0, wqk1)
        wv_b = singles.tile([128, CB, 64], BF16)
        nc.gpsimd.tensor_copy(wv_b[:, :, :], wv_s[:, :, :])

        def body():
            # q^T for both batches: qk0 rows 0:64 = q0, qk1 rows 64:128 = q1
            qk0 = qkp.tile([128, T], BF16, tag="qk0")
            qk1 = qkp.tile([128, T], BF16, tag="qk1")
            # k^T for both batches: rows 0:64 = k0, rows 64:128 = k1
            skk = qkp.tile([128, T], BF16, tag="skk")
            vn0 = vnp.tile([128, TT, 65], BF16, tag="vn0")
            vn1 = vnp.tile([128, TT, 65], BF16, tag="vn1")
            nc.vector.memset(vn0[:, :, 64], 1.0)
            nc.vector.memset(vn1[:, :, 64], 1.0)
            qkt = (qk0, qk1)
            vnt = (vn0, vn1)

            def phase_ab(blk):
                xTs = []
                for b in range(BPC):
                    xT = xTp.tile([128, CB, 512], BF16, tag=f"xT{b}")
                    xTs.append(xT)
                    xt = xp.tile([128, 4, C], BF16, tag="x")
                    base = blk * 512
                    if blk == 0 and b == 0:
                        with tc.high_priority(offset=1 << 20):
                            nc.gpsimd.dma_start(
                                out=xt[:, 0:2, :],
                                in_=x_d[b, base:base + 256, :].rearrange(
                                    "(f p) c -> p f c", p=128))
                        with tc.high_priority():
                            nc.gpsimd.dma_start(
                                out=xt[:, 2:4, :],
                                in_=x_d[b, base + 256:base + 512, :].rearrange(
                                    "(f p) c -> p f c", p=128))
                    else:
                        nc.gpsimd.dma_start(
                            out=xt[:, :, :],
                            in_=x_d[b, base:base + 512, :].rearrange(
                                "(f p) c -> p f c", p=128))
                    for t4 in range(4):
                        ptr = ps_ab.tile([128, CB, 128], BF16, tag="ab")
                        for ci in range(CB):
                            nc.tensor.matmul(ptr[:, ci, :],
                                             xt[:, t4, ci * 128:(ci + 1) * 128],
                                             ident[:, :], is_transpose=True)
                        nc.vector.tensor_copy(
                            xT[:, :, t4 * 128:(t4 + 1) * 128], ptr[:, :, :])
                sl = slice(blk * 512, (blk + 1) * 512)
                # Projections with split-K: each 128-row chunk becomes lo/hi
                # 64-row matmuls at tile positions (0,0)/(64,0) accumulating
                # into two separate PSUM banks (measured 2.2x the K=128
                # stream rate); a DVE add fuses the partials straight to
                # bf16. v partials borrow the oa banks (idle during AB) so
                # the ab ring never waits on the adds.
                for b in range(BPC):
                    if qsplit:
                        pqA = ps_ab.tile([128, 512], F32, tag="ab")
                        pqB = ps_ab.tile([128, 512], F32, tag="ab")
                        for ci in range(CB):
                            nc.tensor.matmul(pqA[:, :], wqk[b][0:64, ci, :],
                                             xTs[b][0:64, ci, :],
                                             start=(ci == 0),
                                             stop=(ci == CB - 1),
                                             tile_position=(0, 0))
                            nc.tensor.matmul(pqB[:, :], wqk[b][64:128, ci, :],
                                             xTs[b][64:128, ci, :],
                                             start=(ci == 0),
                                             stop=(ci == CB - 1),
                                             tile_position=(64, 0))
                        # walrus rejects dual-PSUM-input DVE ops: stage A
                        # to the SBUF destination, then add B in place
                        nc.vector.tensor_copy(qkt[b][:, sl], pqA[:, :])
                        nc.vector.scalar_tensor_tensor(
                            out=qkt[b][:, sl], in0=qkt[b][:, sl], scalar=0.0,
                            in1=pqB[:, :], op0=mybir.AluOpType.add,
                            op1=mybir.AluOpType.add)
                    else:
                        pq = ps_ab.tile([128, 512], F32, tag="ab")
                        for ci in range(CB):
                            nc.tensor.matmul(pq[:, :], wqk[b][:, ci, :],
                                             xTs[b][:, ci, :],
                                             start=(ci == 0),
                                             stop=(ci == CB - 1))
                        nc.vector.tensor_copy(qkt[b][:, sl], pq[:, :])
                    if vsplit:
                        # v in natural [t, h]: same split, partials in the
                        # oa banks (idle during AB)
                        pvvA = ps_oa.tile([128, 4, 64], F32, tag="oa0")
                        pvvB = ps_oa.tile([128, 4, 64], F32, tag="oa1")
                        for t4 in range(4):
                            ts = slice(t4 * 128, (t4 + 1) * 128)
                            for ci in range(CB):
                                nc.tensor.matmul(pvvA[:, t4, :],
                                                 xTs[b][0:64, ci, ts],
                                                 wv_b[0:64, ci, :],
                                                 start=(ci == 0),
                                                 stop=(ci == CB - 1),
                                                 tile_position=(0, 0),
                                                 skip_group_check=True)
                                nc.tensor.matmul(pvvB[:, t4, :],
                                                 xTs[b][64:128, ci, ts],
                                                 wv_b[64:128, ci, :],
                                                 start=(ci == 0),
                                                 stop=(ci == CB - 1),
                                                 tile_position=(64, 0),
                                                 skip_group_check=True)
                        vsl = vnt[b][:, blk * 4:(blk + 1) * 4, 0:64]
                        nc.vector.tensor_copy(vsl, pvvA[:, :, :])
                        nc.vector.scalar_tensor_tensor(
                            out=vsl, in0=vsl, scalar=0.0,
                            in1=pvvB[:, :, :], op0=mybir.AluOpType.add,
                            op1=mybir.AluOpType.add)
                    else:
                        pvv = ps_ab.tile([128, 4, 64], F32, tag="ab")
                        for t4 in range(4):
                            ts = slice(t4 * 128, (t4 + 1) * 128)
                            for ci in range(CB):
                                nc.tensor.matmul(pvv[:, t4, :],
                                                 xTs[b][:, ci, ts],
                                                 wv_b[:, ci, :],
                                                 start=(ci == 0),
                                                 stop=(ci == CB - 1))
                        nc.vector.tensor_copy(
                            vnt[b][:, blk * 4:(blk + 1) * 4, 0:64],
                            pvv[:, :, :])
                # k partition shifts into the shared k tile
                nc.sync.dma_start(out=skk[0:64, sl], in_=qk0[64:128, sl])
                nc.sync.dma_start(out=skk[64:128, sl], in_=qk1[0:64, sl])

            def phase_c(bi):
                # one full 2KB bank per batch: a single bank-wide PSUM
                # accumulation group (start on the very first matmul into the
                # bank, stop on the last). Regions zero lazily on first
                # touch; interleaved per-region start/stops would corrupt
                # neighbours (pending-zero arms the whole 2KB zero region).
                oacc0 = ps_oa.tile([128, 4, 128], F32, tag="oa0")
                oacc1 = ps_oa.tile([128, 4, 128], F32, tag="oa1")
                oaccs = (oacc0, oacc1)
                last = 4 * bi + 3

                def geom(j):
                    r = j - 4 * bi
                    w, c0 = (512, 0) if r <= 0 else (512 - 128 * r, 128 * r)
                    return r, w, c0

                def emit_qk(j):
                    r, w, c0 = geom(j)
                    js = slice(j * 128, (j + 1) * 128)
                    cs = slice(bi * 512 + c0, (bi + 1) * 512)
                    sab = ps_att.tile([128, BPC, 512], F32, tag="att")
                    nc.tensor.matmul(sab[:, 0, 0:w], skk[0:64, js],
                                     qk0[0:64, cs], start=True, stop=True,
                                     tile_position=(0, 0))
                    nc.tensor.matmul(sab[:, 1, 0:w], skk[64:128, js],
                                     qk1[64:128, cs], start=True, stop=True,
                                     tile_position=(64, 0))
                    return sab

                sab = emit_qk(0)
                for j in range(last + 1):
                    r, w, c0 = geom(j)
                    pt = ptp.tile([128, BPC, 512], BF16, tag="pt")
                    nc.scalar.activation(pt[:, :, 0:w], sab[:, :, 0:w],
                                         mybir.ActivationFunctionType.Exp,
                                         scale=SCALE)
                    if r >= 0:
                        nc.gpsimd.affine_select(
                            out=pt[:, :, 0:128], in_=pt[:, :, 0:128],
                            compare_op=mybir.AluOpType.is_ge, fill=0.0,
                            base=0, pattern=[[0, BPC], [1, 128]],
                            channel_multiplier=-1)
                    if j < last:
                        sab = emit_qk(j + 1)
                    # PV: stationary = scores tile (s x t), moving = v|ones
                    for tau in range(max(r, 0), 4):
                        p0 = tau * 128 - c0
                        for b in range(BPC):
                            nc.tensor.matmul(oaccs[b][:, tau, 0:65],
                                             pt[:, b, p0:p0 + 128],
                                             vnt[b][:, j, :],
                                             start=(j == 0 and tau == max(r, 0)),
                                             stop=(j == last and tau == 3),
                                             skip_group_check=True)
                for b in range(BPC):
                    ot = fin.tile([128, 4, 65], F32, tag="ot")
                    if bi == NB - 1 and b == 1:
                        nc.scalar.copy(ot[:, :, :], oaccs[b][:, :, 0:65])
                    else:
                        nc.vector.tensor_copy(ot[:, :, :], oaccs[b][:, :, 0:65])
                    yt = fin.tile([128, 4, 64], F32, tag="yt")
                    if bi < NB - 1:
                        for t4 in range(4):
                            nc.gpsimd.normalize_recip(yt[:, t4, :],
                                                      ot[:, t4, 0:64],
                                                      ot[:, t4, 64:65])
                    else:
                        linv = fin.tile([128, 4], F32, tag="linv")
                        nc.vector.reciprocal(linv[:, :], ot[:, :, 64])
                        for t4 in range(4):
                            nc.vector.tensor_scalar_mul(yt[:, t4, :],
                                                        ot[:, t4, 0:64],
                                                        linv[:, t4:t4 + 1])
                    nc.sync.dma_start(
                        out=y_d[b, bi * 512:(bi + 1) * 512, :].rearrange(
                            "(f p) h -> p f h", p=128),
                        in_=yt[:, :, :])

            for blk in range(NB):
                phase_ab(blk)
                phase_c(blk)

        if reps == 1:
            body()
        else:
            with tc.For_i(0, reps, 1):
                body()

    nc.compile()
    return nc


_CACHE = {}


def _get_program(**kw):
    key = tuple(sorted(kw.items()))
    if key not in _CACHE:
        kw2 = dict(kw)
        v = kw2.pop("v", 2)
        fn = build_program_v2 if int(v) == 2 else build_program
        _CACHE[key] = fn(**kw2)
    return _CACHE[key]


def run_sharded(x, Wq, Wk, Wv, trace=False, **build_kw):
    """Run on 8 cores, return (y_full, BassKernelResults)."""
    nc = _get_program(**build_kw)
    x = np.ascontiguousarray(np.asarray(x, dtype=np.float32))
    Wq = np.ascontiguousarray(np.asarray(Wq, dtype=np.float32))
    Wk = np.ascontiguousarray(np.asarray(Wk, dtype=np.float32))
    Wv = np.ascontiguousarray(np.asarray(Wv, dtype=np.float32))
    xs = x.reshape(NCORES, BPC, T, C)
    in_maps = [{"x": np.ascontiguousarray(xs[i]), "Wq": Wq, "Wk": Wk, "Wv": Wv}
               for i in range(NCORES)]
    res = run_bass_kernel_spmd(nc, in_maps, list(range(NCORES)), trace=trace)
    y = np.stack([res.results[i]["y"] for i in range(NCORES)], axis=0)
    return y.reshape(B, T, H), res


def kernel(x, Wq, Wk, Wv):
    y, _ = run_sharded(x, Wq, Wk, Wv, trace=False)
    return y


# ---------------- timing support (no NTFF profiler in this container) ----


def make_runner(nc, n_iter=1):
    """Build a reusable sharded jit callable for `nc` (mirrors
    bass2jax.run_bass_via_pjrt's multi-core path, without donation so
    device inputs can be reused across timed calls)."""
    import jax
    from jax.sharding import Mesh, PartitionSpec
    try:
        from jax.experimental.shard_map import shard_map
    except ImportError:  # newer jax
        from jax.shard_map import shard_map
    from concourse import bass2jax
    bass2jax.install_neuronx_cc_hook()

    part_name = (nc.partition_id_tensor.name if nc.partition_id_tensor
                 else None)
    in_names, out_names, out_avals, zero_outs = [], [], [], []
    for alloc in nc.m.functions[0].allocations:
        if not isinstance(alloc, mybir.MemoryLocationSet):
            continue
        name = alloc.memorylocations[0].name
        if alloc.kind == "ExternalInput":
            if name != part_name:
                in_names.append(name)
        elif alloc.kind == "ExternalOutput":
            out_names.append(name)
            shape = tuple(alloc.tensor_shape)
            dtype = mybir.dt.np(alloc.dtype)
            out_avals.append(jax.core.ShapedArray(shape, dtype))
            zero_outs.append(np.zeros(shape, dtype))
    n_params = len(in_names)
    all_names = in_names + out_names
    if part_name is not None:
        all_names = all_names + [part_name]

    def _body(*args):
        ins = list(args[:n_params])
        youts = list(args[n_params:n_params + len(out_names)])
        for _ in range(n_iter):
            operands = ins + youts
            if part_name is not None:
                operands.append(bass2jax.partition_id_tensor())
            outs = bass2jax._bass_exec_p.bind(
                *operands, out_avals=tuple(out_avals),
                in_names=tuple(all_names), out_names=tuple(out_names),
                lowering_input_output_aliases=(),
                sim_require_finite=True, sim_require_nnan=True, nc=nc)
            youts = list(outs)
        return tuple(youts)

    devices = jax.devices()[:NCORES]
    mesh = Mesh(np.asarray(devices), ("core",))
    in_specs = (PartitionSpec("core"),) * (n_params + len(out_names))
    out_specs = (PartitionSpec("core"),) * len(out_names)
    fn = jax.jit(shard_map(_body, mesh=mesh, in_specs=in_specs,
                           out_specs=out_specs, check_rep=False),
                 keep_unused=True)
    return fn, in_names, zero_outs, mesh


def _timed_calls(fn, dev_in, iters):
    import time as _time
    import jax
    out = fn(*dev_in)
    jax.block_until_ready(out)
    ts = []
    for _ in range(iters):
        t0 = _time.perf_counter_ns()
        out = fn(*dev_in)
        jax.block_until_ready(out)
        ts.append(_time.perf_counter_ns() - t0)
    ts.sort()
    return ts


def time_calls(nc, in_maps, iters=10):
    """Sorted wall times (ns) of warm sharded calls of nc's NEFF."""
    import jax
    from jax.sharding import NamedSharding, PartitionSpec
    fn, in_names, zero_outs, mesh = make_runner(nc, n_iter=1)
    sh = NamedSharding(mesh, PartitionSpec("core"))
    concat = [np.concatenate([np.asarray(m[n]) for m in in_maps], axis=0)
              for n in in_names]
    concat += [np.zeros((NCORES * z.shape[0], *z.shape[1:]), z.dtype)
               for z in zero_outs]
    dev_in = [jax.device_put(a, sh) for a in concat]
    return _timed_calls(fn, dev_in, iters)


_BASELINE = {}


def baseline_nc():
    """Tiny kernel to measure the axon dispatch floor."""
    if "nc" in _BASELINE:
        return _BASELINE["nc"]
    nc = bacc.Bacc("TRN2", target_bir_lowering=False, debug=False,
                   num_devices=NCORES)
    a = nc.dram_tensor("a", [128, 128], F32, kind="ExternalInput").ap()
    b = nc.dram_tensor("b", [128, 128], F32, kind="ExternalOutput").ap()
    with tile.TileContext(nc) as tc:
        with tc.tile_pool(name="p", bufs=1) as pool:
            t = pool.tile([128, 128], F32)
            nc.sync.dma_start(out=t[:, :], in_=a)
            nc.sync.dma_start(out=b, in_=t[:, :])
    nc.compile()
    _BASELINE["nc"] = nc
    return nc



# revision 17
# speedup vs baseline: 1.0931x; 1.0094x over previous
"""Single-head causal attention on 8 trn2 NeuronCores.

Problem: x[16, 2048, 1024] fp32, Wq/Wk/Wv[1024, 64] fp32 ->
         out[16, 2048, 64] = softmax(causal(q k^T / sqrt(64))) v

Sharding: data-parallel over batch B=16 -> 2 batches per core, no
collectives. SPMD program; the two per-core batches are processed in
lockstep so small-dim matmuls can be packed across them.

v2 structure (build_program_v2, the default; ~96us/rep sustained vs
~114us for v1 on the same protocol, REPS=257 slope):
  * x loaded with gpsimd casting DMAs (fp32 HBM -> bf16 SBUF); x^T via
    PE transposes (1 cyc/row bf16), wide DVE copy-outs.
  * Projections: per-batch packed [Wq|Wk]/[Wk|Wq] -> q^T/k^T; k halves
    partition-shifted into a shared skk tile by SBUF->SBUF DMA.
  * v computed directly in NATURAL [T, 64] layout: stationary = xT
    chunk, moving = Wv (64-col streams run at full rate; weight reloads
    hide under streams). No v^T->v transposes.
  * Scores S^T: two batches' K=64 matmuls concurrent via row tiling
    (0,0)/(64,0) -- genuine 2x since each only half-fills the array.
  * PV swapped: stationary = exp-scores tile [s,t-tile], moving =
    v|ones [s,65] -> output lands NATURAL [t, 64+denominator]: no
    finalize transposes; normalize is a per-partition scalar divide.
  * PV accumulation: ONE bank-wide PSUM group per (batch, block)
    (start on first matmul into the bank, stop on last). HW pending-
    zero arms whole 2KB zero-regions; interleaved per-region groups in
    a shared bank corrupt each other (measured + interp-confirmed).
  * Causal mask: affine_select on Pool over the first 128 columns of
    diagonal tiles only.

Floor analysis: PE MAC floor = QK 32.8k + V 16.4k + scores 17.4k
(row-tile paired) + PV 17.7k + transposes 27.6k ~= 112k cycles; the
sustained PE clock is ~1.2GHz (short bursts boost to ~2.4-2.8GHz,
which is why REPS=33 slopes are unstable; use REPS=257).
Measured dead ends: fp8e4 DoubleRow runs SLOWER than bf16 on HW
despite the cost model 0.5 cyc/row; split-K pairing (K=128 -> 2x64 in
two banks) is a wash (pair concurrency just recovers the full-array
rate, total columns unchanged); 4-way 32-row tiling worse than 2-way;
dual-PSUM-input scalar_tensor_tensor crashes walrus; same-region
accumulation across tile positions faults the device.
"""

import sys

sys.path.insert(0, "/opt/trn_rl_repo")

import numpy as np

import concourse.bass as bass  # noqa: F401
import concourse.bacc as bacc
import concourse.mybir as mybir
import concourse.tile as tile
from concourse.masks import make_identity
from concourse.bass_utils import run_bass_kernel_spmd

B, T, C, H = 16, 2048, 1024, 64
NCORES = 8
BPC = B // NCORES  # batches per core
CB = C // 128      # 8 contraction chunks
TT = T // 128      # 16 T tiles of 128
NB = T // 512      # 4 T blocks of 512
F32 = mybir.dt.float32
BF16 = mybir.dt.bfloat16
SCALE = float(H) ** -0.5


def build_program(reps=1, att_bufs=2, ab_bufs=2, pt_bufs=3, x_bufs=3,
                  xt_bufs=2, oex_bufs=2):
    from contextlib import ExitStack

    nc = bacc.Bacc("TRN2", target_bir_lowering=False, debug=False,
                   num_devices=NCORES)
    x_d = nc.dram_tensor("x", [BPC, T, C], F32, kind="ExternalInput").ap()
    wq_d = nc.dram_tensor("Wq", [C, H], F32, kind="ExternalInput").ap()
    wk_d = nc.dram_tensor("Wk", [C, H], F32, kind="ExternalInput").ap()
    wv_d = nc.dram_tensor("Wv", [C, H], F32, kind="ExternalInput").ap()
    y_d = nc.dram_tensor("y", [BPC, T, H], F32, kind="ExternalOutput").ap()

    with tile.TileContext(nc) as tc, ExitStack() as ctx:
        singles = ctx.enter_context(tc.tile_pool(name="singles", bufs=1))
        xp = ctx.enter_context(tc.tile_pool(name="xp", bufs=x_bufs))
        xTp = ctx.enter_context(tc.tile_pool(name="xTp", bufs=xt_bufs))
        qkp = ctx.enter_context(tc.tile_pool(name="qkp", bufs=1))
        vnp = ctx.enter_context(tc.tile_pool(name="vnp", bufs=1))
        vTp = ctx.enter_context(tc.tile_pool(name="vTp", bufs=2))
        ptp = ctx.enter_context(tc.tile_pool(name="ptp", bufs=pt_bufs))
        oxp = ctx.enter_context(tc.tile_pool(name="oxp", bufs=oex_bufs))
        fin = ctx.enter_context(tc.tile_pool(name="fin", bufs=2))
        ps_att = ctx.enter_context(tc.tile_pool(name="psatt", bufs=att_bufs,
                                                space="PSUM"))
        ps_oa = ctx.enter_context(tc.tile_pool(name="psoa", bufs=1,
                                               space="PSUM"))
        ps_ab = ctx.enter_context(tc.tile_pool(name="psab", bufs=ab_bufs,
                                               space="PSUM"))

        # identity build is dependency-free Pool work; run it before the x
        # descriptor generation so the first transposes aren't blocked on it
        with tc.high_priority():
            ident = singles.tile([128, 128], BF16)
            make_identity(nc, ident[:, :])
            identf = singles.tile([128, 128], F32)
            make_identity(nc, identf[:, :])
            # warm the PE array (HAM clock ramp) with dummy matmuls while
            # the first x transfer is in flight; they borrow an attention
            # ring slot that isn't needed until ~13us in
            warm = ps_att.tile([128, BPC, 512], F32, tag="att")
            for i in range(24):
                nc.tensor.matmul(warm[:, i % 2, 0:128], ident[:, :],
                                 ident[:, :], start=True, stop=True)

        # Weights: fp32 staging via HWDGE (keeps Pool free for the x casting
        # DMAs at startup), then convert/pack on gpsimd. b0 packs [Wq|Wk],
        # b1 [Wk|Wq] so q lands on the batch's own row half with a plain
        # copy.
        wq_s = singles.tile([128, CB, 64], F32)
        wk_s = singles.tile([128, CB, 64], F32)
        wv_s = singles.tile([128, CB, 64], F32)
        nc.sync.dma_start(out=wq_s[:, :, :],
                          in_=wq_d.rearrange("(c p) h -> p c h", p=128))
        nc.sync.dma_start(out=wk_s[:, :, :],
                          in_=wk_d.rearrange("(c p) h -> p c h", p=128))
        nc.sync.dma_start(out=wv_s[:, :, :],
                          in_=wv_d.rearrange("(c p) h -> p c h", p=128))
        # packing on Pool: these wait on the weight transfers, and on DVE
        # they would head-of-line block the first transpose copies
        wqk0 = singles.tile([128, CB, 128], BF16)
        nc.gpsimd.tensor_copy(wqk0[:, :, 0:64], wq_s[:, :, :])
        nc.gpsimd.tensor_copy(wqk0[:, :, 64:128], wk_s[:, :, :])
        wqk1 = singles.tile([128, CB, 128], BF16)
        nc.gpsimd.tensor_copy(wqk1[:, :, 0:64], wk_s[:, :, :])
        nc.gpsimd.tensor_copy(wqk1[:, :, 64:128], wq_s[:, :, :])
        wv2 = singles.tile([128, CB, 128], BF16)
        nc.gpsimd.tensor_copy(wv2[:, :, 0:64], wv_s[:, :, :])
        nc.gpsimd.tensor_copy(wv2[:, :, 64:128], wv_s[:, :, :])
        wqk = (wqk0, wqk1)

        def body():
            # q^T for both batches: qk0 rows 0:64 = q0, qk1 rows 64:128 = q1
            qk0 = qkp.tile([128, T], BF16, tag="qk0")
            qk1 = qkp.tile([128, T], BF16, tag="qk1")
            # k^T for both batches: rows 0:64 = k0, rows 64:128 = k1
            skk = qkp.tile([128, T], BF16, tag="skk")
            vn0 = vnp.tile([128, TT, 65], BF16, tag="vn0")
            vn1 = vnp.tile([128, TT, 65], BF16, tag="vn1")
            nc.vector.memset(vn0[:, :, 64], 1.0)
            nc.vector.memset(vn1[:, :, 64], 1.0)
            qkt = (qk0, qk1)
            vnt = (vn0, vn1)

            def phase_ab(blk):
                """Load + transpose x, projections for T block blk."""
                xTs = []
                for b in range(BPC):
                    xT = xTp.tile([128, CB, 512], BF16, tag=f"xT{b}")
                    xTs.append(xT)
                    # one casting DMA per (batch, block): 1 MB fp32 -> bf16.
                    # The very first block is split in two so the transpose
                    # pipeline fills sooner.
                    xt = xp.tile([128, 4, C], BF16, tag="x")
                    base = blk * 512
                    if blk == 0 and b == 0:
                        # first load split in two and boosted ahead of
                        # everything (even the identity build: it hides
                        # under the transfer) so the pipeline fills asap
                        with tc.high_priority(offset=1 << 20):
                            nc.gpsimd.dma_start(
                                out=xt[:, 0:2, :],
                                in_=x_d[b, base:base + 256, :].rearrange(
                                    "(f p) c -> p f c", p=128))
                        with tc.high_priority():
                            nc.gpsimd.dma_start(
                                out=xt[:, 2:4, :],
                                in_=x_d[b, base + 256:base + 512, :].rearrange(
                                    "(f p) c -> p f c", p=128))
                    else:
                        nc.gpsimd.dma_start(
                            out=xt[:, :, :],
                            in_=x_d[b, base:base + 512, :].rearrange(
                                "(f p) c -> p f c", p=128))
                    for t4 in range(4):
                        ptr = ps_ab.tile([128, CB, 128], BF16, tag="ab")
                        for ci in range(CB):
                            nc.tensor.matmul(ptr[:, ci, :],
                                             xt[:, t4, ci * 128:(ci + 1) * 128],
                                             ident[:, :], is_transpose=True)
                        nc.vector.tensor_copy(
                            xT[:, :, t4 * 128:(t4 + 1) * 128], ptr[:, :, :])
                sl = slice(blk * 512, (blk + 1) * 512)
                for b in range(BPC):
                    pq = ps_ab.tile([128, 512], F32, tag="ab")
                    for ci in range(CB):
                        nc.tensor.matmul(pq[:, :], wqk[b][:, ci, :],
                                         xTs[b][:, ci, :],
                                         start=(ci == 0), stop=(ci == CB - 1))
                    nc.vector.tensor_copy(qkt[b][:, sl], pq[:, :])
                for b in range(BPC):
                    chunks.append(lambda b=b: proj(b))

                def shifts():
                    nc.sync.dma_start(out=skk[0:64, sl], in_=qk0[64:128, sl])
                    nc.sync.dma_start(out=skk[64:128, sl], in_=qk1[0:64, sl])

                chunks.append(shifts)
                return chunks

            def emit_all(chunks):
                for ch in chunks:
                    ch()
                # v for both batches, column-tiled into one PSUM bank
                pv = ps_ab.tile([128, 512], F32, tag="ab")
                for ci in range(CB):
                    nc.tensor.matmul(pv[0:64, :], wv2[:, ci, 0:64],
                                     xTs[0][:, ci, :], start=(ci == 0),
                                     stop=(ci == CB - 1), tile_position=(0, 0))
                    nc.tensor.matmul(pv[64:128, :], wv2[:, ci, 64:128],
                                     xTs[1][:, ci, :], start=(ci == 0),
                                     stop=(ci == CB - 1), tile_position=(0, 64))
                vT = vTp.tile([128, 512], BF16, tag="vT")
                nc.vector.tensor_copy(vT[:, :], pv[:, :])
                for b in range(BPC):
                    pvn = ps_ab.tile([128, 4, 64], BF16, tag="ab")
                    bs = slice(b * 64, (b + 1) * 64)
                    for t4 in range(4):
                        nc.tensor.matmul(
                            pvn[:, t4, :],
                            vT[bs, t4 * 128:(t4 + 1) * 128],
                            ident[bs, bs], is_transpose=True)
                    nc.vector.tensor_copy(
                        vnt[b][:, blk * 4:(blk + 1) * 4, 0:64], pvn[:, :, :])

            def phase_c(bi):
                """Attention for T-row block bi (needs k/v blocks <= bi)."""
                oacc = ps_oa.tile([65, BPC, 512], F32, tag="oa")
                last = 4 * bi + 3

                def geom(j):
                    r = j - 4 * bi
                    w, c0 = (512, 0) if r <= 0 else (512 - 128 * r, 128 * r)
                    return r, w, c0

                def emit_qk(j):
                    r, w, c0 = geom(j)
                    js = slice(j * 128, (j + 1) * 128)
                    cs = slice(bi * 512 + c0, (bi + 1) * 512)
                    sab = ps_att.tile([128, BPC, 512], F32, tag="att")
                    nc.tensor.matmul(sab[:, 0, 0:w], skk[0:64, js],
                                     qk0[0:64, cs], start=True, stop=True,
                                     tile_position=(0, 0))
                    nc.tensor.matmul(sab[:, 1, 0:w], skk[64:128, js],
                                     qk1[64:128, cs], start=True, stop=True,
                                     tile_position=(64, 0))
                    return sab

                # software-pipelined: QK(j+1) is emitted before PV(j) so the
                # PE queue never has a PV (waiting on exp) ahead of ready QK
                sab = emit_qk(0)
                for j in range(last + 1):
                    r, w, c0 = geom(j)
                    pt = ptp.tile([128, BPC, 512], BF16, tag="pt")
                    nc.scalar.activation(pt[:, :, 0:w], sab[:, :, 0:w],
                                         mybir.ActivationFunctionType.Exp,
                                         scale=SCALE)
                    if r >= 0:
                        # keep where within-tile free idx >= partition idx.
                        # Since partitions only span 0..127, columns >= 128
                        # always pass: mask just the first 128 columns.
                        nc.gpsimd.affine_select(
                            out=pt[:, :, 0:128], in_=pt[:, :, 0:128],
                            compare_op=mybir.AluOpType.is_ge, fill=0.0,
                            base=0, pattern=[[0, BPC], [1, 128]],
                            channel_multiplier=-1)
                    if j < last:
                        sab = emit_qk(j + 1)
                    for b in range(BPC):
                        nc.tensor.matmul(oacc[:, b, c0:512], vnt[b][:, j, :],
                                         pt[:, b, 0:w], start=(j == 0),
                                         stop=(j == last))
                for b in range(BPC):
                    oex = oxp.tile([65, 512], F32, tag="oex")
                    # on the last block ACT is idle after its final exp; give
                    # it batch 1's drain so the two finalize chains overlap
                    if bi == NB - 1 and b == 1:
                        nc.scalar.copy(oex[:, :], oacc[:, b, :])
                    else:
                        nc.vector.tensor_copy(oex[:, :], oacc[:, b, :])
                    # NOTE: must NOT share the "ab" ring — that would make
                    # block k+1's transposes wait on this finalize
                    pso = ps_att.tile([128, 4, 65], F32, tag="att")
                    for t4 in range(4):
                        nc.tensor.matmul(pso[:, t4, :],
                                         oex[0:65, t4 * 128:(t4 + 1) * 128],
                                         identf[0:65, 0:65],
                                         is_transpose=True)
                    ot = fin.tile([128, 4, 65], F32, tag="ot")
                    if bi == NB - 1 and b == 1:
                        nc.scalar.copy(ot[:, :, :], pso[:, :, :])
                    else:
                        nc.vector.tensor_copy(ot[:, :, :], pso[:, :, :])
                    yt = fin.tile([128, 4, 64], F32, tag="yt")
                    if bi < NB - 1:
                        for t4 in range(4):
                            # out = ot / l on gpsimd; overwrites the l
                            # column with its reciprocal (unused afterwards)
                            nc.gpsimd.normalize_recip(yt[:, t4, :],
                                                      ot[:, t4, 0:64],
                                                      ot[:, t4, 64:65])
                    else:
                        # last block: Pool's serial ISA ops would sit on the
                        # kernel tail; DVE is idle there
                        linv = fin.tile([128, 4], F32, tag="linv")
                        nc.vector.reciprocal(linv[:, :], ot[:, :, 64])
                        for t4 in range(4):
                            nc.vector.tensor_scalar_mul(yt[:, t4, :],
                                                        ot[:, t4, 0:64],
                                                        linv[:, t4:t4 + 1])
                    nc.sync.dma_start(
                        out=y_d[b, bi * 512:(bi + 1) * 512, :].rearrange(
                            "(f p) h -> p f h", p=128),
                        in_=yt[:, :, :])

            for blk in range(NB):
                phase_ab(blk)
                phase_c(blk)

        if reps == 1:
            body()
        else:
            with tc.For_i(0, reps, 1):
                body()

    nc.compile()
    return nc


def build_program_v2(reps=1, att_bufs=2, ab_bufs=2, pt_bufs=3, x_bufs=3,
                     xt_bufs=2, qsplit=False, vsplit=False):
    """v2: natural-layout V (stationary=xT tile, moving=Wv) and swapped PV
    (stationary=scores tile, moving=v|ones) so attention output lands in
    natural [T, H] layout with the softmax denominator as a free 65th
    column -> no vn/finalize transposes. Scores stay bf16 row-tiled pairs
    (fp8 DoubleRow measured SLOWER than bf16 on HW despite the cost model's
    0.5 cyc/row: 333.8ns vs 213.4ns per row-tiled pair)."""
    from contextlib import ExitStack

    nc = bacc.Bacc("TRN2", target_bir_lowering=False, debug=False,
                   num_devices=NCORES)
    x_d = nc.dram_tensor("x", [BPC, T, C], F32, kind="ExternalInput").ap()
    wq_d = nc.dram_tensor("Wq", [C, H], F32, kind="ExternalInput").ap()
    wk_d = nc.dram_tensor("Wk", [C, H], F32, kind="ExternalInput").ap()
    wv_d = nc.dram_tensor("Wv", [C, H], F32, kind="ExternalInput").ap()
    y_d = nc.dram_tensor("y", [BPC, T, H], F32, kind="ExternalOutput").ap()

    with tile.TileContext(nc) as tc, ExitStack() as ctx:
        singles = ctx.enter_context(tc.tile_pool(name="singles", bufs=1))
        xp = ctx.enter_context(tc.tile_pool(name="xp", bufs=x_bufs))
        xTp = ctx.enter_context(tc.tile_pool(name="xTp", bufs=xt_bufs))
        qkp = ctx.enter_context(tc.tile_pool(name="qkp", bufs=1))
        vnp = ctx.enter_context(tc.tile_pool(name="vnp", bufs=1))
        ptp = ctx.enter_context(tc.tile_pool(name="ptp", bufs=pt_bufs))
        fin = ctx.enter_context(tc.tile_pool(name="fin", bufs=2))
        ps_att = ctx.enter_context(tc.tile_pool(name="psatt", bufs=att_bufs,
                                                space="PSUM"))
        ps_oa = ctx.enter_context(tc.tile_pool(name="psoa", bufs=1,
                                               space="PSUM"))
        ps_ab = ctx.enter_context(tc.tile_pool(name="psab", bufs=ab_bufs,
                                               space="PSUM"))

        with tc.high_priority():
            ident = singles.tile([128, 128], BF16)
            make_identity(nc, ident[:, :])
            # PE warmup during the first x transfer
            warm = ps_att.tile([128, BPC, 512], F32, tag="att")
            for i in range(24):
                nc.tensor.matmul(warm[:, i % 2, 0:128], ident[:, :],
                                 ident[:, :], start=True, stop=True)

        # fp32 weight staging via HWDGE, pack on Pool. b0 packs [Wq|Wk],
        # b1 [Wk|Wq] so q lands on the batch's own row half with a plain
        # copy (same as v1).
        wq_s = singles.tile([128, CB, 64], F32)
        wk_s = singles.tile([128, CB, 64], F32)
        wv_s = singles.tile([128, CB, 64], F32)
        nc.sync.dma_start(out=wq_s[:, :, :],
                          in_=wq_d.rearrange("(c p) h -> p c h", p=128))
        nc.sync.dma_start(out=wk_s[:, :, :],
                          in_=wk_d.rearrange("(c p) h -> p c h", p=128))
        nc.sync.dma_start(out=wv_s[:, :, :],
                          in_=wv_d.rearrange("(c p) h -> p c h", p=128))
        wqk0 = singles.tile([128, CB, 128], BF16)
        nc.gpsimd.tensor_copy(wqk0[:, :, 0:64], wq_s[:, :, :])
        nc.gpsimd.tensor_copy(wqk0[:, :, 64:128], wk_s[:, :, :])
        wqk1 = singles.tile([128, CB, 128], BF16)
        nc.gpsimd.tensor_copy(wqk1[:, :, 0:64], wk_s[:, :, :])
        nc.gpsimd.tensor_copy(wqk1[:, :, 64:128], wq_s[:, :, :])
        wqk = (wqk0, wqk1)
        wv_b = singles.tile([128, CB, 64], BF16)
        nc.gpsimd.tensor_copy(wv_b[:, :, :], wv_s[:, :, :])

        def body():
            # q^T for both batches: qk0 rows 0:64 = q0, qk1 rows 64:128 = q1
            qk0 = qkp.tile([128, T], BF16, tag="qk0")
            qk1 = qkp.tile([128, T], BF16, tag="qk1")
            # k^T for both batches: rows 0:64 = k0, rows 64:128 = k1
            skk = qkp.tile([128, T], BF16, tag="skk")
            vn0 = vnp.tile([128, TT, 65], BF16, tag="vn0")
            vn1 = vnp.tile([128, TT, 65], BF16, tag="vn1")
            nc.vector.memset(vn0[:, :, 64], 1.0)
            nc.vector.memset(vn1[:, :, 64], 1.0)
            qkt = (qk0, qk1)
            vnt = (vn0, vn1)

            def phase_ab_chunks(blk):
                """Emission chunks for block blk's load/transpose/projection
                work. Returned closures are emitted in order, optionally
                interleaved into the previous block's attention j-loop so the
                in-order PE queue has ready work during exp waits."""
                chunks = []
                xTs = []
                xts = []
                for b in range(BPC):
                    xTs.append(xTp.tile([128, CB, 512], BF16, tag=f"xT{b}",
                                        name=f"xT{b}"))
                    xts.append(xp.tile([128, 4, C], BF16, tag="x", name="xt"))

                def load(b):
                    xt = xts[b]
                    base = blk * 512
                    if blk == 0 and b == 0:
                        with tc.high_priority(offset=1 << 20):
                            nc.gpsimd.dma_start(
                                out=xt[:, 0:2, :],
                                in_=x_d[b, base:base + 256, :].rearrange(
                                    "(f p) c -> p f c", p=128))
                        with tc.high_priority():
                            nc.gpsimd.dma_start(
                                out=xt[:, 2:4, :],
                                in_=x_d[b, base + 256:base + 512, :].rearrange(
                                    "(f p) c -> p f c", p=128))
                    else:
                        nc.gpsimd.dma_start(
                            out=xt[:, :, :],
                            in_=x_d[b, base:base + 512, :].rearrange(
                                "(f p) c -> p f c", p=128))

                def tgroup(b, t4):
                    ptr = ps_ab.tile([128, CB, 128], BF16, tag="ab",
                                     name="ptr")
                    for ci in range(CB):
                        nc.tensor.matmul(ptr[:, ci, :],
                                         xts[b][:, t4, ci * 128:(ci + 1) * 128],
                                         ident[:, :], is_transpose=True)
                    nc.vector.tensor_copy(
                        xTs[b][:, :, t4 * 128:(t4 + 1) * 128], ptr[:, :, :])

                for b in range(BPC):
                    chunks.append(lambda b=b: load(b))
                    for t4 in range(4):
                        chunks.append(lambda b=b, t4=t4: tgroup(b, t4))
                sl = slice(blk * 512, (blk + 1) * 512)
                # Projections with split-K: each 128-row chunk becomes lo/hi
                # 64-row matmuls at tile positions (0,0)/(64,0) accumulating
                # into two separate PSUM banks (measured 2.2x the K=128
                # stream rate); a DVE add fuses the partials straight to
                # bf16. v partials borrow the oa banks (idle during AB) so
                # the ab ring never waits on the adds.
                def proj(b):
                    if qsplit:
                        pqA = ps_ab.tile([128, 512], F32, tag="ab")
                        pqB = ps_ab.tile([128, 512], F32, tag="ab")
                        for ci in range(CB):
                            nc.tensor.matmul(pqA[:, :], wqk[b][0:64, ci, :],
                                             xTs[b][0:64, ci, :],
                                             start=(ci == 0),
                                             stop=(ci == CB - 1),
                                             tile_position=(0, 0))
                            nc.tensor.matmul(pqB[:, :], wqk[b][64:128, ci, :],
                                             xTs[b][64:128, ci, :],
                                             start=(ci == 0),
                                             stop=(ci == CB - 1),
                                             tile_position=(64, 0))
                        # walrus rejects dual-PSUM-input DVE ops: stage A
                        # to the SBUF destination, then add B in place
                        nc.vector.tensor_copy(qkt[b][:, sl], pqA[:, :])
                        nc.vector.scalar_tensor_tensor(
                            out=qkt[b][:, sl], in0=qkt[b][:, sl], scalar=0.0,
                            in1=pqB[:, :], op0=mybir.AluOpType.add,
                            op1=mybir.AluOpType.add)
                    else:
                        pq = ps_ab.tile([128, 512], F32, tag="ab")
                        for ci in range(CB):
                            nc.tensor.matmul(pq[:, :], wqk[b][:, ci, :],
                                             xTs[b][:, ci, :],
                                             start=(ci == 0),
                                             stop=(ci == CB - 1))
                        nc.vector.tensor_copy(qkt[b][:, sl], pq[:, :])
                    if vsplit:
                        # v in natural [t, h]: same split, partials in the
                        # oa banks (idle during AB)
                        pvvA = ps_oa.tile([128, 4, 64], F32, tag="oa0")
                        pvvB = ps_oa.tile([128, 4, 64], F32, tag="oa1")
                        for t4 in range(4):
                            ts = slice(t4 * 128, (t4 + 1) * 128)
                            for ci in range(CB):
                                nc.tensor.matmul(pvvA[:, t4, :],
                                                 xTs[b][0:64, ci, ts],
                                                 wv_b[0:64, ci, :],
                                                 start=(ci == 0),
                                                 stop=(ci == CB - 1),
                                                 tile_position=(0, 0),
                                                 skip_group_check=True)
                                nc.tensor.matmul(pvvB[:, t4, :],
                                                 xTs[b][64:128, ci, ts],
                                                 wv_b[64:128, ci, :],
                                                 start=(ci == 0),
                                                 stop=(ci == CB - 1),
                                                 tile_position=(64, 0),
                                                 skip_group_check=True)
                        vsl = vnt[b][:, blk * 4:(blk + 1) * 4, 0:64]
                        nc.vector.tensor_copy(vsl, pvvA[:, :, :])
                        nc.vector.scalar_tensor_tensor(
                            out=vsl, in0=vsl, scalar=0.0,
                            in1=pvvB[:, :, :], op0=mybir.AluOpType.add,
                            op1=mybir.AluOpType.add)
                    else:
                        pvv = ps_ab.tile([128, 4, 64], F32, tag="ab")
                        for t4 in range(4):
                            ts = slice(t4 * 128, (t4 + 1) * 128)
                            for ci in range(CB):
                                nc.tensor.matmul(pvv[:, t4, :],
                                                 xTs[b][:, ci, ts],
                                                 wv_b[:, ci, :],
                                                 start=(ci == 0),
                                                 stop=(ci == CB - 1))
                        nc.vector.tensor_copy(
                            vnt[b][:, blk * 4:(blk + 1) * 4, 0:64],
                            pvv[:, :, :])
                for b in range(BPC):
                    chunks.append(lambda b=b: proj(b))

                def shifts():
                    nc.sync.dma_start(out=skk[0:64, sl], in_=qk0[64:128, sl])
                    nc.sync.dma_start(out=skk[64:128, sl], in_=qk1[0:64, sl])

                chunks.append(shifts)
                return chunks

            def emit_all(chunks):
                for ch in chunks:
                    ch()

            def phase_c(bi, nxt):
                # one full 2KB bank per batch: a single bank-wide PSUM
                # accumulation group (start on the very first matmul into the
                # bank, stop on the last). Regions zero lazily on first
                # touch; interleaved per-region start/stops would corrupt
                # neighbours (pending-zero arms the whole 2KB zero region).
                oacc0 = ps_oa.tile([128, 4, 128], F32, tag="oa0")
                oacc1 = ps_oa.tile([128, 4, 128], F32, tag="oa1")
                oaccs = (oacc0, oacc1)
                last = 4 * bi + 3

                def geom(j):
                    r = j - 4 * bi
                    w, c0 = (512, 0) if r <= 0 else (512 - 128 * r, 128 * r)
                    return r, w, c0

                def emit_qk(j):
                    r, w, c0 = geom(j)
                    js = slice(j * 128, (j + 1) * 128)
                    cs = slice(bi * 512 + c0, (bi + 1) * 512)
                    sab = ps_att.tile([128, BPC, 512], F32, tag="att")
                    nc.tensor.matmul(sab[:, 0, 0:w], skk[0:64, js],
                                     qk0[0:64, cs], start=True, stop=True,
                                     tile_position=(0, 0))
                    nc.tensor.matmul(sab[:, 1, 0:w], skk[64:128, js],
                                     qk1[64:128, cs], start=True, stop=True,
                                     tile_position=(64, 0))
                    return sab

                sab = emit_qk(0)
                for j in range(last + 1):
                    r, w, c0 = geom(j)
                    pt = ptp.tile([128, BPC, 512], BF16, tag="pt")
                    nc.scalar.activation(pt[:, :, 0:w], sab[:, :, 0:w],
                                         mybir.ActivationFunctionType.Exp,
                                         scale=SCALE)
                    if r >= 0:
                        nc.gpsimd.affine_select(
                            out=pt[:, :, 0:128], in_=pt[:, :, 0:128],
                            compare_op=mybir.AluOpType.is_ge, fill=0.0,
                            base=0, pattern=[[0, BPC], [1, 128]],
                            channel_multiplier=-1)
                    if j < last:
                        sab = emit_qk(j + 1)
                    # interleave the next block's load/transpose/projection
                    # chunks here: their PE work fills the exp(j) wait that
                    # would otherwise stall the in-order PE queue before PV(j)
                    take = -(-len(nxt) // (last + 1 - j))
                    for _ in range(take):
                        nxt.pop(0)()
                    # PV: stationary = scores tile (s x t), moving = v|ones
                    for tau in range(max(r, 0), 4):
                        p0 = tau * 128 - c0
                        for b in range(BPC):
                            nc.tensor.matmul(oaccs[b][:, tau, 0:65],
                                             pt[:, b, p0:p0 + 128],
                                             vnt[b][:, j, :],
                                             start=(j == 0 and tau == max(r, 0)),
                                             stop=(j == last and tau == 3),
                                             skip_group_check=True)
                for b in range(BPC):
                    ot = fin.tile([128, 4, 65], F32, tag="ot")
                    if bi == NB - 1 and b == 1:
                        nc.scalar.copy(ot[:, :, :], oaccs[b][:, :, 0:65])
                    else:
                        nc.vector.tensor_copy(ot[:, :, :], oaccs[b][:, :, 0:65])
                    yt = fin.tile([128, 4, 64], F32, tag="yt")
                    if bi < NB - 1:
                        for t4 in range(4):
                            nc.gpsimd.normalize_recip(yt[:, t4, :],
                                                      ot[:, t4, 0:64],
                                                      ot[:, t4, 64:65])
                    else:
                        linv = fin.tile([128, 4], F32, tag="linv")
                        nc.vector.reciprocal(linv[:, :], ot[:, :, 64])
                        for t4 in range(4):
                            nc.vector.tensor_scalar_mul(yt[:, t4, :],
                                                        ot[:, t4, 0:64],
                                                        linv[:, t4:t4 + 1])
                    nc.sync.dma_start(
                        out=y_d[b, bi * 512:(bi + 1) * 512, :].rearrange(
                            "(f p) h -> p f h", p=128),
                        in_=yt[:, :, :])

            emit_all(phase_ab_chunks(0))
            for bi in range(NB):
                phase_c(bi, phase_ab_chunks(bi + 1) if bi + 1 < NB else [])

        if reps == 1:
            body()
        else:
            with tc.For_i(0, reps, 1):
                body()

    nc.compile()
    return nc


_CACHE = {}


def _get_program(**kw):
    key = tuple(sorted(kw.items()))
    if key not in _CACHE:
        kw2 = dict(kw)
        v = kw2.pop("v", 2)
        fn = build_program_v2 if int(v) == 2 else build_program
        _CACHE[key] = fn(**kw2)
    return _CACHE[key]


def run_sharded(x, Wq, Wk, Wv, trace=False, **build_kw):
    """Run on 8 cores, return (y_full, BassKernelResults)."""
    nc = _get_program(**build_kw)
    x = np.ascontiguousarray(np.asarray(x, dtype=np.float32))
    Wq = np.ascontiguousarray(np.asarray(Wq, dtype=np.float32))
    Wk = np.ascontiguousarray(np.asarray(Wk, dtype=np.float32))
    Wv = np.ascontiguousarray(np.asarray(Wv, dtype=np.float32))
    xs = x.reshape(NCORES, BPC, T, C)
    in_maps = [{"x": np.ascontiguousarray(xs[i]), "Wq": Wq, "Wk": Wk, "Wv": Wv}
               for i in range(NCORES)]
    res = run_bass_kernel_spmd(nc, in_maps, list(range(NCORES)), trace=trace)
    y = np.stack([res.results[i]["y"] for i in range(NCORES)], axis=0)
    return y.reshape(B, T, H), res


def kernel(x, Wq, Wk, Wv):
    y, _ = run_sharded(x, Wq, Wk, Wv, trace=False)
    return y


# ---------------- timing support (no NTFF profiler in this container) ----


def make_runner(nc, n_iter=1):
    """Build a reusable sharded jit callable for `nc` (mirrors
    bass2jax.run_bass_via_pjrt's multi-core path, without donation so
    device inputs can be reused across timed calls)."""
    import jax
    from jax.sharding import Mesh, PartitionSpec
    try:
        from jax.experimental.shard_map import shard_map
    except ImportError:  # newer jax
        from jax.shard_map import shard_map
    from concourse import bass2jax
    bass2jax.install_neuronx_cc_hook()

    part_name = (nc.partition_id_tensor.name if nc.partition_id_tensor
                 else None)
    in_names, out_names, out_avals, zero_outs = [], [], [], []
    for alloc in nc.m.functions[0].allocations:
        if not isinstance(alloc, mybir.MemoryLocationSet):
            continue
        name = alloc.memorylocations[0].name
        if alloc.kind == "ExternalInput":
            if name != part_name:
                in_names.append(name)
        elif alloc.kind == "ExternalOutput":
            out_names.append(name)
            shape = tuple(alloc.tensor_shape)
            dtype = mybir.dt.np(alloc.dtype)
            out_avals.append(jax.core.ShapedArray(shape, dtype))
            zero_outs.append(np.zeros(shape, dtype))
    n_params = len(in_names)
    all_names = in_names + out_names
    if part_name is not None:
        all_names = all_names + [part_name]

    def _body(*args):
        ins = list(args[:n_params])
        youts = list(args[n_params:n_params + len(out_names)])
        for _ in range(n_iter):
            operands = ins + youts
            if part_name is not None:
                operands.append(bass2jax.partition_id_tensor())
            outs = bass2jax._bass_exec_p.bind(
                *operands, out_avals=tuple(out_avals),
                in_names=tuple(all_names), out_names=tuple(out_names),
                lowering_input_output_aliases=(),
                sim_require_finite=True, sim_require_nnan=True, nc=nc)
            youts = list(outs)
        return tuple(youts)

    devices = jax.devices()[:NCORES]
    mesh = Mesh(np.asarray(devices), ("core",))
    in_specs = (PartitionSpec("core"),) * (n_params + len(out_names))
    out_specs = (PartitionSpec("core"),) * len(out_names)
    fn = jax.jit(shard_map(_body, mesh=mesh, in_specs=in_specs,
                           out_specs=out_specs, check_rep=False),
                 keep_unused=True)
    return fn, in_names, zero_outs, mesh


def _timed_calls(fn, dev_in, iters):
    import time as _time
    import jax
    out = fn(*dev_in)
    jax.block_until_ready(out)
    ts = []
    for _ in range(iters):
        t0 = _time.perf_counter_ns()
        out = fn(*dev_in)
        jax.block_until_ready(out)
        ts.append(_time.perf_counter_ns() - t0)
    ts.sort()
    return ts


def time_calls(nc, in_maps, iters=10):
    """Sorted wall times (ns) of warm sharded calls of nc's NEFF."""
    import jax
    from jax.sharding import NamedSharding, PartitionSpec
    fn, in_names, zero_outs, mesh = make_runner(nc, n_iter=1)
    sh = NamedSharding(mesh, PartitionSpec("core"))
    concat = [np.concatenate([np.asarray(m[n]) for m in in_maps], axis=0)
              for n in in_names]
    concat += [np.zeros((NCORES * z.shape[0], *z.shape[1:]), z.dtype)
               for z in zero_outs]
    dev_in = [jax.device_put(a, sh) for a in concat]
    return _timed_calls(fn, dev_in, iters)


_BASELINE = {}


def baseline_nc():
    """Tiny kernel to measure the axon dispatch floor."""
    if "nc" in _BASELINE:
        return _BASELINE["nc"]
    nc = bacc.Bacc("TRN2", target_bir_lowering=False, debug=False,
                   num_devices=NCORES)
    a = nc.dram_tensor("a", [128, 128], F32, kind="ExternalInput").ap()
    b = nc.dram_tensor("b", [128, 128], F32, kind="ExternalOutput").ap()
    with tile.TileContext(nc) as tc:
        with tc.tile_pool(name="p", bufs=1) as pool:
            t = pool.tile([128, 128], F32)
            nc.sync.dma_start(out=t[:, :], in_=a)
            nc.sync.dma_start(out=b, in_=t[:, :])
    nc.compile()
    _BASELINE["nc"] = nc
    return nc

